# revision 1
# baseline (speedup 1.0000x reference)
"""BiLSTM+CRF NLL loss kernel for 8 Trainium2 NeuronCores (v2).

Sharding: data-parallel on batch (32 sequences per core). Each core runs the
full BiLSTM + emission + CRF forward/backward partition recurrences for its
shard; host combines per-core partials into the scalar loss.

v2 vs v1:
- embedding gather interleaved into the LSTM phase (was 0.52ms serial prefix)
- gate order [g,f,i,o]: tanh(g) leads the scalar queue; sigmoid split into
  (f,i) and (o) so the o-gate leaves the critical path
- bulk (x-part) matmuls spread one-per-step instead of bursts
- CRF transition matmuls in bf16 (one PE pass instead of two fp32 passes)
- CRF renorm decoupled from the recurrence chain (scale applied 3 hops late)
"""

import numpy as np
import ml_dtypes

import concourse.bass as bass
import concourse.tile as tile
from concourse import mybir
from concourse.bass_utils import run_bass_kernel_spmd

F32 = mybir.dt.float32
BF16 = mybir.dt.bfloat16

B, S, V, I, NB = 256, 512, 30000, 100, 19
BOS, EOS = 17, 18
NCORES = 8
BC = B // NCORES          # 32 sequences per core
NT = BC * S               # 16384 tokens per core
KP = I + 1                # 101: embedding dims + ones row (bias aug)
EPAD = 128                # padded embedding row length
RENORM = 16               # CRF renorm interval
TBLK = 4                  # steps per PSUM gate block
GCH = 8                   # gate chunks: (gamma in [g,f,i,o]) x (dir in [f,b])

_CACHE = {}


def _build_nc(s_len=S):
    SL = s_len
    NTL = BC * SL
    NBLK = SL // TBLK
    NCHUNK = NTL // 128

    nc = bass.Bass()

    # ---- dram I/O ----
    emb_d = nc.dram_tensor("emb_pad", [V, EPAD], BF16, kind="ExternalInput")
    idx_d = nc.dram_tensor("idxs", [128, NTL // 128], mybir.dt.int32, kind="ExternalInput")
    eye_d = nc.dram_tensor("eye", [128, 128], BF16, kind="ExternalInput")
    wih_d = nc.dram_tensor("wih", [128, GCH, 128], BF16, kind="ExternalInput")
    whh_d = nc.dram_tensor("whh", [128, GCH, 128], BF16, kind="ExternalInput")
    wc_d = nc.dram_tensor("wc", [128, 2, NB], BF16, kind="ExternalInput")
    bc_d = nc.dram_tensor("bc", [NB, 1], F32, kind="ExternalInput")
    esm_d = nc.dram_tensor("esm", [NB, NB], BF16, kind="ExternalInput")
    est_d = nc.dram_tensor("est", [NB, NB], BF16, kind="ExternalInput")
    etb_d = nc.dram_tensor("etb", [NB, 1], F32, kind="ExternalInput")
    veb_d = nc.dram_tensor("veb", [NB, BC], F32, kind="ExternalInput")
    ones19_d = nc.dram_tensor("ones19", [NB, 1], BF16, kind="ExternalInput")
    ones19f_d = nc.dram_tensor("ones19f", [NB, 1], F32, kind="ExternalInput")
    one1x19_d = nc.dram_tensor("one1x19", [1, NB], BF16, kind="ExternalInput")

    y_out = nc.dram_tensor("y_out", [NB, NTL], BF16, kind="ExternalOutput")
    res_out = nc.dram_tensor("res", [4, BC], F32, kind="ExternalOutput")

    SIG = mybir.ActivationFunctionType.Sigmoid
    TANH = mybir.ActivationFunctionType.Tanh
    EXP = mybir.ActivationFunctionType.Exp
    LOG = mybir.ActivationFunctionType.Ln

    with tile.TileContext(nc) as tc:
        with tc.tile_pool(name="big", bufs=1) as bp:
            xeT_f = bp.tile([128, NTL], BF16, tag="xeT_f")
            eye_s = bp.tile([128, 128], BF16, tag="eye_s")
            # h storage: col (t+1)*32 = h after step t; col 0 = h(-1)=0
            h_all = bp.tile([128, 2, NTL + BC], BF16, tag="h_all")
            Y = bp.tile([NB, NTL], BF16, tag="Y")
            idx_f = bp.tile([128, NTL // 128], mybir.dt.int32, tag="idx_f")
            wih = bp.tile([128, GCH, 128], BF16, tag="wih")
            whh = bp.tile([128, GCH, 128], BF16, tag="whh")
            wc = bp.tile([128, 2, NB], BF16, tag="wc")
            bc_s = bp.tile([NB, 1], F32, tag="bc_s")
            esm = bp.tile([NB, NB], BF16, tag="esm")
            est = bp.tile([NB, NB], BF16, tag="est")
            etb = bp.tile([NB, 1], F32, tag="etb")
            veb = bp.tile([NB, BC], F32, tag="veb")
            ones19 = bp.tile([NB, 1], BF16, tag="ones19")
            ones19f = bp.tile([NB, 1], F32, tag="ones19f")
            one1x19 = bp.tile([1, NB], BF16, tag="one1x19")
            gates_s0 = bp.tile([128, GCH, BC], F32, tag="gates_s0")
            gates_s1 = bp.tile([128, GCH, BC], F32, tag="gates_s1")
            cell = bp.tile([128, 6, BC], F32, tag="cell")  # [c | gA | gB]
            th0 = bp.tile([128, 2, BC], F32, tag="th0")
            th1 = bp.tile([128, 2, BC], F32, tag="th1")
            u_0 = bp.tile([128, 4, BC], F32, tag="u_0")
            u_1 = bp.tile([128, 4, BC], F32, tag="u_1")
            Wf0 = bp.tile([NB, BC], BF16, tag="Wf0")
            Wf1 = bp.tile([NB, BC], BF16, tag="Wf1")
            Vb0 = bp.tile([NB, BC], BF16, tag="Vb0")
            Vb1 = bp.tile([NB, BC], BF16, tag="Vb1")
            P2 = bp.tile([NB, BC], F32, tag="P2")
            acc_f = bp.tile([1, BC], F32, tag="acc_f")
            acc_b = bp.tile([1, BC], F32, tag="acc_b")
            rec_f = bp.tile([1, BC], F32, tag="rec_f")
            rec_fb = bp.tile([1, BC], BF16, tag="rec_fb")
            rec_b = bp.tile([1, BC], F32, tag="rec_b")
            rec_bb = bp.tile([1, BC], BF16, tag="rec_bb")
            lg_f = bp.tile([1, BC], F32, tag="lg_f")
            lg_b = bp.tile([1, BC], F32, tag="lg_b")
            res_s = bp.tile([4, BC], F32, tag="res_s")

            # ---- small loads ----
            nc.sync.dma_start(out=idx_f[:, :], in_=idx_d[:])
            nc.sync.dma_start(out=eye_s[:, :], in_=eye_d[:])
            nc.sync.dma_start(out=wih[:, :, :], in_=wih_d[:])
            nc.sync.dma_start(out=whh[:, :, :], in_=whh_d[:])
            nc.sync.dma_start(out=wc[:, :, :], in_=wc_d[:])
            nc.sync.dma_start(out=bc_s[:, :], in_=bc_d[:])
            nc.sync.dma_start(out=esm[:, :], in_=esm_d[:])
            nc.sync.dma_start(out=est[:, :], in_=est_d[:])
            nc.sync.dma_start(out=etb[:, :], in_=etb_d[:])
            nc.sync.dma_start(out=veb[:, :], in_=veb_d[:])
            nc.sync.dma_start(out=ones19[:, :], in_=ones19_d[:])
            nc.sync.dma_start(out=ones19f[:, :], in_=ones19f_d[:])
            nc.sync.dma_start(out=one1x19[:, :], in_=one1x19_d[:])

            nc.vector.memset(acc_f[:, :], 0.0)
            nc.vector.memset(acc_b[:, :], 0.0)

            # =========== phase A: gather + both LSTMs, interleaved ==========
            # one nat tile per chunk: gather DMAs then carry no pool-WAR
            # waits, so no multi-us gpsimd Drains throttle the gather
            with tc.tile_pool(name="gat", bufs=NCHUNK) as gp, \
                 tc.tile_pool(name="gps", bufs=2, space="PSUM") as gpp, \
                 tc.tile_pool(name="psA", bufs=1, space="PSUM") as pa:
                GA = pa.tile([128, GCH, TBLK, BC], F32, tag="GA")
                GB = pa.tile([128, GCH, TBLK, BC], F32, tag="GB")
                gbuf = (GA, GB)

                nat_tiles = {}

                def gather_dma(c):
                    nat = gp.tile([128, EPAD], BF16, tag="nat")
                    nc.gpsimd.indirect_dma_start(
                        out=nat[:, :], out_offset=None,
                        in_=emb_d[:, :],
                        in_offset=bass.IndirectOffsetOnAxis(
                            ap=idx_f[:, c:c + 1], axis=0),
                    )
                    nat_tiles[c] = nat

                def gather_tp(c):
                    nat = nat_tiles.pop(c)
                    tp = gpp.tile([128, 128], BF16, tag="tp")
                    nc.tensor.transpose(tp[:, :], nat[:, :], eye_s[:, :])
                    nc.vector.tensor_copy(
                        xeT_f[:, c * 128:(c + 1) * 128], tp[:, :])

                xe3 = xeT_f[0:KP, :].rearrange("p (t b) -> p t b", b=BC)

                def bulk_mm(k, c):
                    G = gbuf[k % 2]
                    if c % 2 == 0:
                        rhs = xe3[:, k * TBLK:(k + 1) * TBLK, :]
                    else:
                        hi = SL - 1 - k * TBLK
                        rhs = (xe3[:, hi:hi - TBLK:-1, :]
                               if hi - TBLK >= 0 else xe3[:, hi::-1, :])
                    nc.tensor.matmul(
                        G[:, c, :, :], wih[0:KP, c, :], rhs,
                        start=True, stop=False, skip_group_check=True,
                    )

                def step(t):
                    G = gbuf[(t // TBLK) % 2]
                    tau = t % TBLK
                    rd = t * BC
                    gs = gates_s0 if t % 2 == 0 else gates_s1
                    th = th0 if t % 2 == 0 else th1
                    gofs = 2 + 2 * (t % 2)  # ghat half in cell
                    if t > 0:
                        # recurrent matmuls: f,i first, then g, then o
                        for c in (0, 1, 2, 3, 4, 5):
                            d = c % 2
                            nc.tensor.matmul(
                                G[:, c, tau, :], whh[0:I, c, :],
                                h_all[0:I, d, rd:rd + BC],
                                start=False, stop=True, skip_group_check=True,
                            )
                    # sigmoid(f, i) first (longer op overlaps the mm tail)
                    nc.scalar.activation(gs[:, 0:4, :], G[:, 0:4, tau, :], SIG)
                    # tanh(g) second
                    nc.scalar.activation(
                        cell[:, gofs:gofs + 2, :].rearrange("p a b -> p (a b)"),
                        G[:, 4:6, tau, :], TANH)
                    if t > 0:
                        for c in (6, 7):
                            d = c % 2
                            nc.tensor.matmul(
                                G[:, c, tau, :], whh[0:I, c, :],
                                h_all[0:I, d, rd:rd + BC],
                                start=False, stop=True, skip_group_check=True,
                            )
                    # sigmoid(o) off the critical path
                    nc.scalar.activation(gs[:, 6:8, :], G[:, 6:8, tau, :], SIG)
                    if t % 2 == 0:
                        cpair = cell[:, 0:4, :].rearrange("p a b -> p (a b)")
                    else:
                        cpair = cell[:, :, :].rearrange(
                            "p (x y) b -> p x (y b)", x=3)[:, 0::2, :]
                    u_t = u_0 if t % 2 == 0 else u_1
                    if t > 0:
                        nc.vector.tensor_mul(
                            u_t[:, :, :].rearrange("p a b -> p (a b)"),
                            gs[:, 0:4, :].rearrange("p a b -> p (a b)"),
                            cpair)
                        nc.vector.tensor_add(
                            cell[:, 0:2, :].rearrange("p a b -> p (a b)"),
                            u_t[:, 0:2, :].rearrange("p a b -> p (a b)"),
                            u_t[:, 2:4, :].rearrange("p a b -> p (a b)"))
                    else:
                        nc.vector.tensor_mul(
                            cell[:, 0:2, :].rearrange("p a b -> p (a b)"),
                            gs[:, 2:4, :].rearrange("p a b -> p (a b)"),
                            cell[:, gofs:gofs + 2, :].rearrange("p a b -> p (a b)"))
                    nc.scalar.activation(
                        th[:, :, :].rearrange("p a b -> p (a b)"),
                        cell[:, 0:2, :].rearrange("p a b -> p (a b)"), TANH)
                    wr = (t + 1) * BC
                    nc.vector.tensor_mul(
                        h_all[:, :, wr:wr + BC], gs[:, 6:8, :], th[:, :, :]
                    )

                import os
                _ALLGATHER = bool(int(os.environ.get("KV2_ALLGATHER", "0")))
                if _ALLGATHER:
                    for j in range(NCHUNK):
                        gather_dma(j)
                        gather_tp(j)
                else:
                    # prologue: DMA chunks for blocks 0..5 (both ends,
                    # interleaved so early transposes unblock first),
                    # transpose chunks for blocks 0..3
                    for j in (0, 1, 2, 3, 4, 5):
                        gather_dma(j)
                        gather_dma(NCHUNK - 1 - j)
                    for j in (0, 1, 2, 3):
                        gather_tp(j)
                        gather_tp(NCHUNK - 1 - j)
                for c in range(GCH):
                    bulk_mm(0, c)
                for c in range(GCH):
                    bulk_mm(1, c)

                # chunk c serves fwd block c and bwd block NCHUNK-1-c, so every
                # chunk must be resident before block NBLK//2. DMA the chunk
                # used by block j at block j-6, transpose it at block j-4 —
                # ~2 blocks of runway so transposes never stall PE on gpsimd.
                mid = NCHUNK // 2 - 1  # 63
                for k in range(NBLK):
                    for tau in range(TBLK):
                        step(k * TBLK + tau)
                        # spread helper work across the 4 steps of the block
                        if tau == 0:
                            if not _ALLGATHER and 6 <= k + 6 <= mid:
                                gather_dma(k + 6)
                            if k + 2 < NBLK:
                                bulk_mm(k + 2, 0)
                                bulk_mm(k + 2, 2)
                        elif tau == 1:
                            if not _ALLGATHER and 4 <= k + 4 <= mid:
                                gather_tp(k + 4)
                            if k + 2 < NBLK:
                                bulk_mm(k + 2, 4)
                                bulk_mm(k + 2, 6)
                        elif tau == 2:
                            cb = NCHUNK - 7 - k
                            if not _ALLGATHER and cb >= mid + 1:
                                gather_dma(cb)
                            if k + 2 < NBLK:
                                bulk_mm(k + 2, 1)
                                bulk_mm(k + 2, 3)
                        else:
                            cb = NCHUNK - 5 - k
                            if not _ALLGATHER and mid + 1 <= cb <= NCHUNK - 5:
                                gather_tp(cb)
                            if k + 2 < NBLK:
                                bulk_mm(k + 2, 5)
                                bulk_mm(k + 2, 7)

            # ========= phase B: emissions -> Y = exp(em + bc) ========
            with tc.tile_pool(name="psB", bufs=4, space="PSUM") as pb:
                EBLK = 16  # tokens per emission block
                for blk in range(SL // EBLK):
                    t0 = blk * EBLK
                    em_ps = pb.tile([NB, EBLK * BC], F32, tag="em_ps")
                    # hf for token t lives at col (t+1)*BC
                    nc.tensor.matmul(
                        em_ps[:, :], wc[0:I, 0, :],
                        h_all[0:I, 0, (t0 + 1) * BC:(t0 + 1 + EBLK) * BC],
                        start=True, stop=False, skip_group_check=True,
                    )
                    # hb for token t lives at round (SL-1-t) -> col (SL-t)*BC
                    hb_ap = h_all[0:I, 1, :].rearrange("p (t b) -> p t b", b=BC)
                    nc.tensor.matmul(
                        em_ps[:, :].rearrange("p (t b) -> p t b", b=BC),
                        wc[0:I, 1, :],
                        hb_ap[:, SL - t0:SL - t0 - EBLK:-1, :],
                        start=False, stop=True, skip_group_check=True,
                    )
                    nc.scalar.activation(
                        Y[:, t0 * BC:(t0 + EBLK) * BC], em_ps[:, :], EXP, bias=bc_s[:, 0:1]
                    )

            nc.sync.dma_start(out=y_out[:], in_=Y[:, :])

            # ========= phase C: CRF partition (bf16, off-chain renorm) ======
            with tc.tile_pool(name="psC", bufs=2, space="PSUM") as pc, \
                 tc.tile_pool(name="psC2", bufs=1, space="PSUM") as pc2:
                # W0 = Y_0 * exp(T[BOS,:]) ; V = veb * Y_last
                # chain state double-buffered: hop t reads W[(t-1)%2], writes
                # W[t%2] — the hop mul then has no in-place WAR, collapsing
                # its sync waits to the single PE (matmul) semaphore
                Wfb = (Wf0, Wf1)
                Vbb = (Vb0, Vb1)
                nc.vector.tensor_scalar_mul(Wf0[:, :], Y[0:NB, 0:BC], etb[:, 0:1])
                nc.vector.tensor_mul(Vb0[:, :], veb[:, :],
                                     Y[0:NB, (SL - 1) * BC:SL * BC])

                # small phase-C PSUM tensors: one bank per chain so the fwd
                # and bwd renorm pipelines don't false-serialize on a bank
                crfF = pc2.tile([NB, 3 * BC], F32, tag="crfF")
                crfB = pc2.tile([NB, 2 * BC], F32, tag="crfB")
                rf_ps = crfF[:, 0:BC]
                sf_ps = crfF[0:1, BC:2 * BC]
                dot_ps = crfF[0:1, 2 * BC:3 * BC]
                rb_ps = crfB[:, 0:BC]
                sb_ps = crfB[0:1, BC:2 * BC]

                def renorm_snap(w_sb, s_ps):
                    # s = ones19^T @ w (PE, off the recurrence chain)
                    nc.tensor.matmul(s_ps, ones19[:, :], w_sb[:, :],
                                     skip_group_check=True)

                def renorm_mid(s_ps, rec, recb, r_ps, lg, acc):
                    nc.vector.reciprocal(rec[:, :], s_ps)
                    nc.vector.tensor_copy(recb[:, :], rec[:, :])
                    nc.tensor.matmul(r_ps, one1x19[:, :], recb[:, :],
                                     skip_group_check=True)
                    nc.scalar.activation(lg[:, :], s_ps, LOG)
                    nc.vector.tensor_add(acc[:, :], acc[:, :], lg[:, :])

                wb_prev = None
                HALF = SL // 2
                for r in range(HALF):
                    # forward chain: t = 1..HALF-1
                    if r >= 1:
                        t = r
                        Wsrc, Wdst = Wfb[(t - 1) % 2], Wfb[t % 2]
                        wf_ps = pc.tile([NB, BC], F32, tag="wf_ps")
                        nc.tensor.matmul(wf_ps[:, :], esm[:, :], Wsrc[:, :],
                                         skip_group_check=True)
                        if t % RENORM == 0 and t >= RENORM:
                            renorm_snap(Wsrc, sf_ps)
                        nc.vector.tensor_mul(
                            Wdst[:, :], wf_ps[:, :], Y[0:NB, t * BC:(t + 1) * BC]
                        )
                        if t % RENORM == 0 and t >= RENORM:
                            renorm_mid(sf_ps, rec_f, rec_fb, rf_ps, lg_f, acc_f)
                        if t >= RENORM + 3 and (t - 3) % RENORM == 0:
                            nc.vector.tensor_mul(Wdst[:, :], Wdst[:, :], rf_ps)
                    # backward chain: applications t+1 = SL-1 - r
                    Vsrc, Vdst = Vbb[r % 2], Vbb[(r + 1) % 2]
                    wb_ps = pc.tile([NB, BC], F32, tag="wb_ps")
                    nc.tensor.matmul(wb_ps[:, :], est[:, :], Vsrc[:, :],
                                     skip_group_check=True)
                    if r % RENORM == 0 and r >= RENORM:
                        renorm_snap(Vsrc, sb_ps)
                    if r < HALF - 1:
                        ty = SL - 2 - r  # next Y column for backward chain
                        nc.vector.tensor_mul(
                            Vdst[:, :], wb_ps[:, :], Y[0:NB, ty * BC:(ty + 1) * BC]
                        )
                        if r % RENORM == 0 and r >= RENORM:
                            renorm_mid(sb_ps, rec_b, rec_bb, rb_ps, lg_b, acc_b)
                        if r >= RENORM + 3 and (r - 3) % RENORM == 0:
                            nc.vector.tensor_mul(Vdst[:, :], Vdst[:, :], rb_ps)
                    wb_prev = wb_ps

                # meet at t=HALF-1: P2 = Wf_{HALF-1} * beta_{HALF-1}
                nc.vector.tensor_mul(P2[:, :], Wfb[(HALF - 1) % 2][:, :],
                                     wb_prev[:, :])
                nc.tensor.matmul(dot_ps, ones19f[:, :], P2[:, :],
                                 skip_group_check=True)
                nc.scalar.activation(res_s[0:1, :], dot_ps, LOG)

            nc.sync.dma_start(out=res_out[0:1], in_=res_s[0:1, :])
            nc.sync.dma_start(out=res_out[1:2], in_=acc_f[:, :])
            nc.sync.dma_start(out=res_out[2:3], in_=acc_b[:, :])

    return nc


def _split_waits(nc):
    """Walrus codegen allows ~1 sync-wait on compute instrs; move excess
    waits onto injected same-engine Drain instructions (which allow many)."""
    from concourse import mybir as mb
    n = 0
    for f in nc.m.functions:
        for blk in f.blocks:
            insts = blk.instructions
            new_list = []
            for ins in insts:
                si = ins.sync_info
                if si is not None and si.on_wait and len(si.on_wait) > 1:
                    keep = list(si.on_wait)[-1:] \
                        if type(ins).__name__ == 'InstDrain' else []
                    for w in list(si.on_wait)[:-1] if keep else list(si.on_wait):
                        d = mb.InstDrain(
                            name=f"{ins.name}-ws{n}", ins=[], outs=[])
                        d.engine = ins.engine
                        d.sync_info = mb.SyncInfo(on_wait=[w], on_update=[])
                        new_list.append(d)
                        n += 1
                    ins.sync_info = mb.SyncInfo(
                        on_wait=keep, on_update=list(si.on_update))
                new_list.append(ins)
            del insts[:]
            insts.extend(new_list)
    return n


def _prep_host(inputs):
    emb = np.asarray(inputs["emb"], np.float32)
    T = np.asarray(inputs["transitions"], np.float32)
    W1 = np.asarray(inputs["W1"], np.float32)
    b1 = np.asarray(inputs["b1"], np.float32)
    W2 = np.asarray(inputs["W2"], np.float32)
    b2 = np.asarray(inputs["b2"], np.float32)

    emb_pad = np.zeros((V, EPAD), np.float32)
    emb_pad[:, 0:I] = emb
    emb_pad[:, I] = 1.0  # bias-aug ones row

    # gate reorder: pytorch [i,f,g,o] -> ours [f,i,g,o]
    perm = np.concatenate([np.arange(I, 2 * I), np.arange(0, I),
                           np.arange(2 * I, 3 * I), np.arange(3 * I, 4 * I)])

    def pack_dir(Wih, Whh, bih, bhh):
        Wih, Whh = Wih[perm], Whh[perm]
        bias = (bih + bhh)[perm]
        wih = np.zeros((4, 128, 128), np.float32)  # [gamma, k, m]
        whh = np.zeros((4, 128, 128), np.float32)
        for g in range(4):
            wih[g, 0:I, 0:I] = Wih[g * I:(g + 1) * I].T
            wih[g, I, 0:I] = bias[g * I:(g + 1) * I]
            whh[g, 0:I, 0:I] = Whh[g * I:(g + 1) * I].T
        return wih, whh

    wih_f, whh_f = pack_dir(np.asarray(inputs["Wih_f"], np.float32),
                            np.asarray(inputs["Whh_f"], np.float32),
                            np.asarray(inputs["bih_f"], np.float32),
                            np.asarray(inputs["bhh_f"], np.float32))
    wih_b, whh_b = pack_dir(np.asarray(inputs["Wih_b"], np.float32),
                            np.asarray(inputs["Whh_b"], np.float32),
                            np.asarray(inputs["bih_b"], np.float32),
                            np.asarray(inputs["bhh_b"], np.float32))

    wih = np.zeros((128, GCH, 128), np.float32)
    whh = np.zeros((128, GCH, 128), np.float32)
    for g in range(4):
        wih[:, g * 2 + 0, :] = wih_f[g]
        wih[:, g * 2 + 1, :] = wih_b[g]
        whh[:, g * 2 + 0, :] = whh_f[g]
        whh[:, g * 2 + 1, :] = whh_b[g]

    Wc = W2 @ W1                      # [19, 200]
    bcv = W2 @ b1 + b2                # [19]
    wc = np.zeros((128, 2, NB), np.float32)
    wc[0:I, 0, :] = Wc[:, 0:I].T
    wc[0:I, 1, :] = Wc[:, I:2 * I].T

    c0 = float(np.log(np.sum(np.exp(bcv))))
    esm = np.exp(T - c0)
    est = esm.T.copy()
    etb = np.exp(T[BOS, :]).reshape(NB, 1)
    veb = np.broadcast_to(np.exp(T[:, EOS]).reshape(NB, 1), (NB, BC)).copy()

    bf = ml_dtypes.bfloat16
    common = {
        "emb_pad": emb_pad.astype(bf),
        "wih": wih.astype(bf),
        "whh": whh.astype(bf),
        "wc": wc.astype(bf),
        "bc": bcv.reshape(NB, 1).astype(np.float32),
        "esm": esm.astype(bf),
        "est": est.astype(bf),
        "etb": etb.astype(np.float32),
        "veb": veb.astype(np.float32),
        "ones19": np.ones((NB, 1), bf),
        "ones19f": np.ones((NB, 1), np.float32),
        "one1x19": np.ones((1, NB), bf),
    }
    return common, c0, bcv


def kernel(**inputs):
    x = np.asarray(inputs["x"]).reshape(B, S).astype(np.int64)
    target = np.asarray(inputs["target"]).reshape(B, S).astype(np.int64)
    T = np.asarray(inputs["transitions"], np.float32)

    common, c0, bcv = _prep_host(inputs)

    common["eye"] = np.eye(128, dtype=ml_dtypes.bfloat16)
    in_maps = []
    for c in range(NCORES):
        xs = x[c * BC:(c + 1) * BC]  # [BC, S]
        # fwd token order: col t*BC + b  -> x[b, t]
        idx_fwd = xs.T.reshape(-1).astype(np.int32)
        idxs = idx_fwd.reshape(NT // 128, 128).T.copy()
        in_maps.append({**common, "idxs": idxs})

    if "nc" not in _CACHE:
        nc0 = _build_nc()
        _split_waits(nc0)
        mybir.codegen_inst_isa_subclasses(nc0)
        _CACHE["nc"] = nc0
    nc = _CACHE["nc"]
    _CACHE["in_maps"] = in_maps

    results = run_bass_kernel_spmd(nc, in_maps, list(range(NCORES))).results

    # host combine
    t_sc = (T[target[:, :-1], target[:, 1:]].sum(1)
            + T[BOS, target[:, 0]] + T[target[:, -1], EOS])  # [B]

    losses = np.zeros(B, np.float64)
    for c in range(NCORES):
        yv = np.asarray(results[c]["y_out"], ml_dtypes.bfloat16).astype(np.float32)
        res = np.asarray(results[c]["res"], np.float32)
        logY = np.log(yv).reshape(NB, S, BC)  # log Y = em + bc - c0... (em+bc)
        tg = target[c * BC:(c + 1) * BC]      # [BC, S]
        bi = np.arange(BC)
        e_sc = np.zeros(BC, np.float64)
        for t in range(S):
            e_sc += logY[tg[:, t], t, bi]
        partition = res[0] + res[1] + res[2] + (S - 1) * c0
        losses[c * BC:(c + 1) * BC] = (
            e_sc + t_sc[c * BC:(c + 1) * BC] - partition
        )
    return np.float32(-losses.mean())



# revision 8
# speedup vs baseline: 1.0922x; 1.0922x over previous
"""BiLSTM+CRF NLL loss kernel for 8 Trainium2 NeuronCores (v3).

Sharding: data-parallel on batch (32 sequences per core). Each core runs the
full BiLSTM + emission + CRF forward/backward partition recurrences for its
shard; host combines per-core partials into the scalar loss.

v3 vs v2 (trace: 2551ns/step chain = mm,σ,tanh_g,mul,add,tanh_c,mul_h):
- tanh(g) removed from the serial ACT chain: g pre-acts are scaled 2x on the
  host so one sigmoid instruction covers f,i,g (tanh(g) = 2σ(2g)-1), and the
  cell update becomes c = f*c + 2(σ_g-0.5)*i via fused scalar_tensor_tensor
- cell/gate DVE pipeline in bf16 (2x DVE rate), cell updated in place in a
  fixed tile (no cross-engine WAR)
- CRF fwd/bwd hop multiplies merged into one strided-AP tensor_tensor
"""

import numpy as np
import ml_dtypes

import concourse.bass as bass
import concourse.tile as tile
from concourse import mybir
from concourse.bass_utils import run_bass_kernel_spmd

F32 = mybir.dt.float32
BF16 = mybir.dt.bfloat16

B, S, V, I, NB = 256, 512, 30000, 100, 19
BOS, EOS = 17, 18
NCORES = 8
BC = B // NCORES          # 32 sequences per core
NT = BC * S               # 16384 tokens per core
KP = I + 1                # 101: embedding dims + ones row (bias aug)
EPAD = 128                # padded embedding row length
RENORM = 16               # CRF renorm interval
TBLK = 4                  # steps per PSUM gate block
GCH = 8                   # gate chunks: (gamma in [g,f,i,o]) x (dir in [f,b])

_CACHE = {}


def _build_nc(s_len=S):
    SL = s_len
    NTL = BC * SL
    NBLK = SL // TBLK
    NCHUNK = NTL // 128

    nc = bass.Bass()

    # ---- dram I/O ----
    emb_d = nc.dram_tensor("emb_pad", [V, EPAD], BF16, kind="ExternalInput")
    idx_d = nc.dram_tensor("idxs", [128, NTL // 128], mybir.dt.int32, kind="ExternalInput")
    eye_d = nc.dram_tensor("eye", [128, 128], BF16, kind="ExternalInput")
    wih_d = nc.dram_tensor("wih", [128, GCH, 128], BF16, kind="ExternalInput")
    whh_d = nc.dram_tensor("whh", [128, GCH, 128], BF16, kind="ExternalInput")
    wc_d = nc.dram_tensor("wc", [128, 2, NB], BF16, kind="ExternalInput")
    bc_d = nc.dram_tensor("bc", [NB, 1], F32, kind="ExternalInput")
    esm_d = nc.dram_tensor("esm", [NB, NB], BF16, kind="ExternalInput")
    est_d = nc.dram_tensor("est", [NB, NB], BF16, kind="ExternalInput")
    etb_d = nc.dram_tensor("etb", [NB, 1], F32, kind="ExternalInput")
    veb_d = nc.dram_tensor("veb", [NB, BC], F32, kind="ExternalInput")
    ones19_d = nc.dram_tensor("ones19", [NB, 1], BF16, kind="ExternalInput")
    ones19f_d = nc.dram_tensor("ones19f", [NB, 1], F32, kind="ExternalInput")
    one1x19_d = nc.dram_tensor("one1x19", [1, NB], BF16, kind="ExternalInput")

    y_out = nc.dram_tensor("y_out", [NB, NTL], BF16, kind="ExternalOutput")
    res_out = nc.dram_tensor("res", [4, BC], F32, kind="ExternalOutput")

    SIG = mybir.ActivationFunctionType.Sigmoid
    TANH = mybir.ActivationFunctionType.Tanh
    EXP = mybir.ActivationFunctionType.Exp
    LOG = mybir.ActivationFunctionType.Ln

    with tile.TileContext(nc) as tc:
        with tc.tile_pool(name="big", bufs=1) as bp:
            xeT_f = bp.tile([128, NTL], BF16, tag="xeT_f")
            eye_s = bp.tile([128, 128], BF16, tag="eye_s")
            # h storage: col (t+1)*32 = h after step t; col 0 = h(-1)=0
            h_all = bp.tile([128, 2, NTL + BC], BF16, tag="h_all")
            Y = bp.tile([NB, NTL], BF16, tag="Y")
            idx_f = bp.tile([128, NTL // 128], mybir.dt.int32, tag="idx_f")
            wih = bp.tile([128, GCH, 128], BF16, tag="wih")
            whh = bp.tile([128, GCH, 128], BF16, tag="whh")
            wc = bp.tile([128, 2, NB], BF16, tag="wc")
            bc_s = bp.tile([NB, 1], F32, tag="bc_s")
            esm = bp.tile([NB, NB], BF16, tag="esm")
            est = bp.tile([NB, NB], BF16, tag="est")
            etb = bp.tile([NB, 1], F32, tag="etb")
            veb = bp.tile([NB, BC], F32, tag="veb")
            ones19 = bp.tile([NB, 1], BF16, tag="ones19")
            ones19f = bp.tile([NB, 1], F32, tag="ones19f")
            one1x19 = bp.tile([1, NB], BF16, tag="one1x19")
            gates_s0 = bp.tile([128, GCH, BC], BF16, tag="gates_s0")
            gates_s1 = bp.tile([128, GCH, BC], BF16, tag="gates_s1")
            cellc = bp.tile([128, 2, BC], BF16, tag="cellc")  # c, in-place
            th0 = bp.tile([128, 2, BC], BF16, tag="th0")
            th1 = bp.tile([128, 2, BC], BF16, tag="th1")
            u_f = bp.tile([128, 2, BC], BF16, tag="u_f")
            u_i = bp.tile([128, 2, BC], BF16, tag="u_i")
            # CRF chain state: [parity, chain(0=fwd,1=bwd), BC]
            CH = bp.tile([NB, 2, 2, BC], BF16, tag="CH")
            P2 = bp.tile([NB, BC], F32, tag="P2")
            acc_f = bp.tile([1, BC], F32, tag="acc_f")
            acc_b = bp.tile([1, BC], F32, tag="acc_b")
            rec_f = bp.tile([1, BC], F32, tag="rec_f")
            rec_fb = bp.tile([1, BC], BF16, tag="rec_fb")
            rec_b = bp.tile([1, BC], F32, tag="rec_b")
            rec_bb = bp.tile([1, BC], BF16, tag="rec_bb")
            lg_f = bp.tile([1, BC], F32, tag="lg_f")
            lg_b = bp.tile([1, BC], F32, tag="lg_b")
            res_s = bp.tile([4, BC], F32, tag="res_s")

            # ---- small loads ----
            nc.sync.dma_start(out=idx_f[:, :], in_=idx_d[:])
            nc.sync.dma_start(out=eye_s[:, :], in_=eye_d[:])
            nc.sync.dma_start(out=wih[:, :, :], in_=wih_d[:])
            nc.sync.dma_start(out=whh[:, :, :], in_=whh_d[:])
            nc.sync.dma_start(out=wc[:, :, :], in_=wc_d[:])
            nc.sync.dma_start(out=bc_s[:, :], in_=bc_d[:])
            nc.sync.dma_start(out=esm[:, :], in_=esm_d[:])
            nc.sync.dma_start(out=est[:, :], in_=est_d[:])
            nc.sync.dma_start(out=etb[:, :], in_=etb_d[:])
            nc.sync.dma_start(out=veb[:, :], in_=veb_d[:])
            nc.sync.dma_start(out=ones19[:, :], in_=ones19_d[:])
            nc.sync.dma_start(out=ones19f[:, :], in_=ones19f_d[:])
            nc.sync.dma_start(out=one1x19[:, :], in_=one1x19_d[:])

            nc.vector.memset(acc_f[:, :], 0.0)
            nc.vector.memset(acc_b[:, :], 0.0)

            # =========== phase A: gather + both LSTMs, interleaved ==========
            # one nat tile per chunk: gather DMAs then carry no pool-WAR
            # waits, so no multi-us gpsimd Drains throttle the gather
            with tc.tile_pool(name="gat", bufs=NCHUNK) as gp, \
                 tc.tile_pool(name="gps", bufs=2, space="PSUM") as gpp, \
                 tc.tile_pool(name="psA", bufs=1, space="PSUM") as pa:
                GA = pa.tile([128, GCH, TBLK, BC], F32, tag="GA")
                GB = pa.tile([128, GCH, TBLK, BC], F32, tag="GB")
                gbuf = (GA, GB)

                nat_tiles = {}

                def gather_dma(c):
                    nat = gp.tile([128, EPAD], BF16, tag="nat")
                    nc.gpsimd.indirect_dma_start(
                        out=nat[:, :], out_offset=None,
                        in_=emb_d[:, :],
                        in_offset=bass.IndirectOffsetOnAxis(
                            ap=idx_f[:, c:c + 1], axis=0),
                    )
                    nat_tiles[c] = nat

                def gather_tp(c):
                    nat = nat_tiles.pop(c)
                    tp = gpp.tile([128, 128], BF16, tag="tp")
                    nc.tensor.transpose(tp[:, :], nat[:, :], eye_s[:, :])
                    nc.vector.tensor_copy(
                        xeT_f[:, c * 128:(c + 1) * 128], tp[:, :])

                xe3 = xeT_f[0:KP, :].rearrange("p (t b) -> p t b", b=BC)

                def bulk_mm(k, c):
                    G = gbuf[k % 2]
                    if c % 2 == 0:
                        rhs = xe3[:, k * TBLK:(k + 1) * TBLK, :]
                    else:
                        hi = SL - 1 - k * TBLK
                        rhs = (xe3[:, hi:hi - TBLK:-1, :]
                               if hi - TBLK >= 0 else xe3[:, hi::-1, :])
                    nc.tensor.matmul(
                        G[:, c, :, :], wih[0:KP, c, :], rhs,
                        start=True, stop=False, skip_group_check=True,
                    )

                MULT = mybir.AluOpType.mult
                ADD = mybir.AluOpType.add
                SUB = mybir.AluOpType.subtract

                def step(t):
                    G = gbuf[(t // TBLK) % 2]
                    tau = t % TBLK
                    rd = t * BC
                    gs = gates_s0 if t % 2 == 0 else gates_s1
                    th = th0 if t % 2 == 0 else th1
                    if t > 0:
                        # recurrent matmuls: f,i,g first, then o
                        for c in (0, 1, 2, 3, 4, 5):
                            d = c % 2
                            nc.tensor.matmul(
                                G[:, c, tau, :], whh[0:I, c, :],
                                h_all[0:I, d, rd:rd + BC],
                                start=False, stop=True, skip_group_check=True,
                            )
                    # one sigmoid covers f,i,g (g pre-acts carry a 2x host
                    # scale, so sigma here encodes tanh(g) = 2*sigma(2g)-1)
                    nc.scalar.activation(gs[:, 0:6, :], G[:, 0:6, tau, :], SIG)
                    if t > 0:
                        for c in (6, 7):
                            d = c % 2
                            nc.tensor.matmul(
                                G[:, c, tau, :], whh[0:I, c, :],
                                h_all[0:I, d, rd:rd + BC],
                                start=False, stop=True, skip_group_check=True,
                            )
                    # sigmoid(o) off the critical path
                    nc.scalar.activation(gs[:, 6:8, :], G[:, 6:8, tau, :], SIG)
                    # u_i = (sigma_g - 0.5) * i   [= tanh(g)/2 * i]
                    nc.vector.scalar_tensor_tensor(
                        u_i[:, :, :], gs[:, 4:6, :], 0.5, gs[:, 2:4, :],
                        op0=SUB, op1=MULT)
                    if t > 0:
                        # u_f = f * c ; c = 2*u_i + u_f  (in place)
                        nc.vector.tensor_mul(
                            u_f[:, :, :], gs[:, 0:2, :], cellc[:, :, :])
                        nc.vector.scalar_tensor_tensor(
                            cellc[:, :, :], u_i[:, :, :], 2.0, u_f[:, :, :],
                            op0=MULT, op1=ADD)
                    else:
                        nc.vector.tensor_scalar_mul(
                            cellc[:, :, :], u_i[:, :, :], 2.0)
                    nc.scalar.activation(th[:, :, :], cellc[:, :, :], TANH)
                    wr = (t + 1) * BC
                    nc.vector.tensor_mul(
                        h_all[:, :, wr:wr + BC], gs[:, 6:8, :], th[:, :, :]
                    )

                import os
                _ALLGATHER = bool(int(os.environ.get("KV2_ALLGATHER", "0")))
                if _ALLGATHER:
                    for j in range(NCHUNK):
                        gather_dma(j)
                        gather_tp(j)
                else:
                    # prologue: DMA chunks for blocks 0..5 (both ends,
                    # interleaved so early transposes unblock first),
                    # transpose chunks for blocks 0..3
                    for j in (0, 1, 2, 3, 4, 5):
                        gather_dma(j)
                        gather_dma(NCHUNK - 1 - j)
                    for j in (0, 1, 2, 3):
                        gather_tp(j)
                        gather_tp(NCHUNK - 1 - j)
                for c in range(GCH):
                    bulk_mm(0, c)
                for c in range(GCH):
                    bulk_mm(1, c)

                # chunk c serves fwd block c and bwd block NCHUNK-1-c, so every
                # chunk must be resident before block NBLK//2. DMA the chunk
                # used by block j at block j-6, transpose it at block j-4 —
                # ~2 blocks of runway so transposes never stall PE on gpsimd.
                mid = NCHUNK // 2 - 1  # 63
                for k in range(NBLK):
                    for tau in range(TBLK):
                        step(k * TBLK + tau)
                        # spread helper work across the 4 steps of the block
                        if tau == 0:
                            if not _ALLGATHER and 6 <= k + 6 <= mid:
                                gather_dma(k + 6)
                            if k + 2 < NBLK:
                                bulk_mm(k + 2, 0)
                                bulk_mm(k + 2, 2)
                        elif tau == 1:
                            if not _ALLGATHER and 4 <= k + 4 <= mid:
                                gather_tp(k + 4)
                            if k + 2 < NBLK:
                                bulk_mm(k + 2, 4)
                                bulk_mm(k + 2, 6)
                        elif tau == 2:
                            cb = NCHUNK - 7 - k
                            if not _ALLGATHER and cb >= mid + 1:
                                gather_dma(cb)
                            if k + 2 < NBLK:
                                bulk_mm(k + 2, 1)
                                bulk_mm(k + 2, 3)
                        else:
                            cb = NCHUNK - 5 - k
                            if not _ALLGATHER and mid + 1 <= cb <= NCHUNK - 5:
                                gather_tp(cb)
                            if k + 2 < NBLK:
                                bulk_mm(k + 2, 5)
                                bulk_mm(k + 2, 7)

            # ========= phase B: emissions -> Y = exp(em + bc) ========
            with tc.tile_pool(name="psB", bufs=4, space="PSUM") as pb:
                EBLK = 16  # tokens per emission block
                for blk in range(SL // EBLK):
                    t0 = blk * EBLK
                    em_ps = pb.tile([NB, EBLK * BC], F32, tag="em_ps")
                    # hf for token t lives at col (t+1)*BC
                    nc.tensor.matmul(
                        em_ps[:, :], wc[0:I, 0, :],
                        h_all[0:I, 0, (t0 + 1) * BC:(t0 + 1 + EBLK) * BC],
                        start=True, stop=False, skip_group_check=True,
                    )
                    # hb for token t lives at round (SL-1-t) -> col (SL-t)*BC
                    hb_ap = h_all[0:I, 1, :].rearrange("p (t b) -> p t b", b=BC)
                    nc.tensor.matmul(
                        em_ps[:, :].rearrange("p (t b) -> p t b", b=BC),
                        wc[0:I, 1, :],
                        hb_ap[:, SL - t0:SL - t0 - EBLK:-1, :],
                        start=False, stop=True, skip_group_check=True,
                    )
                    nc.scalar.activation(
                        Y[:, t0 * BC:(t0 + EBLK) * BC], em_ps[:, :], EXP, bias=bc_s[:, 0:1]
                    )

            nc.sync.dma_start(out=y_out[:], in_=Y[:, :])

            # ========= phase C: CRF partition (bf16, off-chain renorm) ======
            with tc.tile_pool(name="psC", bufs=2, space="PSUM") as pc, \
                 tc.tile_pool(name="psC2", bufs=1, space="PSUM") as pc2:
                # W0 = Y_0 * exp(T[BOS,:]) ; V = veb * Y_last
                # chain state in CH[parity, chain, :]: hop r reads parity
                # (r-1)%2, writes r%2 — no in-place WAR; the fwd and bwd hop
                # multiplies merge into ONE strided-AP tensor_tensor per r
                Yp = Y[0:NB, :].rearrange("p (t b) -> p t b", b=BC)
                nc.vector.tensor_scalar_mul(CH[:, 0, 0, :], Y[0:NB, 0:BC],
                                            etb[:, 0:1])
                nc.vector.tensor_mul(CH[:, 1, 1, :], veb[:, :],
                                     Y[0:NB, (SL - 1) * BC:SL * BC])

                # small phase-C PSUM tensors: one bank per chain so the fwd
                # and bwd renorm pipelines don't false-serialize on a bank
                crfF = pc2.tile([NB, 3 * BC], F32, tag="crfF")
                crfB = pc2.tile([NB, 2 * BC], F32, tag="crfB")
                rf_ps = crfF[:, 0:BC]
                sf_ps = crfF[0:1, BC:2 * BC]
                dot_ps = crfF[0:1, 2 * BC:3 * BC]
                rb_ps = crfB[:, 0:BC]
                sb_ps = crfB[0:1, BC:2 * BC]

                def renorm_snap(w_sb, s_ps):
                    # s = ones19^T @ w (PE, off the recurrence chain)
                    nc.tensor.matmul(s_ps, ones19[:, :], w_sb[:, :],
                                     skip_group_check=True)

                def renorm_mid(s_ps, rec, recb, r_ps, lg, acc):
                    nc.vector.reciprocal(rec[:, :], s_ps)
                    nc.vector.tensor_copy(recb[:, :], rec[:, :])
                    nc.tensor.matmul(r_ps, one1x19[:, :], recb[:, :],
                                     skip_group_check=True)
                    nc.scalar.activation(lg[:, :], s_ps, LOG)
                    nc.vector.tensor_add(acc[:, :], acc[:, :], lg[:, :])

                wb_prev = None
                HALF = SL // 2
                for r in range(HALF):
                    p, q = r % 2, (r + 1) % 2  # dst / src parity
                    ty = SL - 2 - r  # next Y column for backward chain
                    wfb_ps = pc.tile([NB, 2, BC], F32, tag="wfb_ps")
                    # backward chain mm (always)
                    nc.tensor.matmul(wfb_ps[:, 1, :], est[:, :],
                                     CH[:, q, 1, :], skip_group_check=True)
                    # forward chain mm: t = r = 1..HALF-1
                    if r >= 1:
                        nc.tensor.matmul(wfb_ps[:, 0, :], esm[:, :],
                                         CH[:, q, 0, :], skip_group_check=True)
                    if r % RENORM == 0 and r >= RENORM:
                        renorm_snap(CH[:, q, 0, :], sf_ps)
                        renorm_snap(CH[:, q, 1, :], sb_ps)
                    # hop multiplies: one strided TT covers both chains
                    if 1 <= r < HALF - 1:
                        nc.vector.tensor_mul(
                            CH[:, p, :, :], wfb_ps[:, :, :],
                            Yp[:, r:ty + 1:(ty - r), :])
                    elif r == 0:
                        nc.vector.tensor_mul(
                            CH[:, p, 1, :], wfb_ps[:, 1, :],
                            Y[0:NB, ty * BC:(ty + 1) * BC])
                    else:  # r == HALF-1: forward hop only
                        nc.vector.tensor_mul(
                            CH[:, p, 0, :], wfb_ps[:, 0, :],
                            Y[0:NB, r * BC:(r + 1) * BC])
                    if r % RENORM == 0 and r >= RENORM:
                        renorm_mid(sf_ps, rec_f, rec_fb, rf_ps, lg_f, acc_f)
                        renorm_mid(sb_ps, rec_b, rec_bb, rb_ps, lg_b, acc_b)
                    if r >= RENORM + 3 and (r - 3) % RENORM == 0:
                        nc.vector.tensor_mul(CH[:, p, 0, :], CH[:, p, 0, :],
                                             rf_ps)
                        nc.vector.tensor_mul(CH[:, p, 1, :], CH[:, p, 1, :],
                                             rb_ps)
                    wb_prev = wfb_ps

                # meet at t=HALF-1: P2 = Wf_{HALF-1} * beta_{HALF-1}
                nc.vector.tensor_mul(P2[:, :], CH[:, (HALF - 1) % 2, 0, :],
                                     wb_prev[:, 1, :])
                nc.tensor.matmul(dot_ps, ones19f[:, :], P2[:, :],
                                 skip_group_check=True)
                nc.scalar.activation(res_s[0:1, :], dot_ps, LOG)

            nc.sync.dma_start(out=res_out[0:1], in_=res_s[0:1, :])
            nc.sync.dma_start(out=res_out[1:2], in_=acc_f[:, :])
            nc.sync.dma_start(out=res_out[2:3], in_=acc_b[:, :])

    return nc


def _split_waits(nc):
    """Walrus codegen allows ~1 sync-wait on compute instrs; move excess
    waits onto injected same-engine Drain instructions (which allow many)."""
    from concourse import mybir as mb
    n = 0
    for f in nc.m.functions:
        for blk in f.blocks:
            insts = blk.instructions
            new_list = []
            for ins in insts:
                si = ins.sync_info
                if si is not None and si.on_wait and len(si.on_wait) > 1:
                    keep = list(si.on_wait)[-1:] \
                        if type(ins).__name__ == 'InstDrain' else []
                    for w in list(si.on_wait)[:-1] if keep else list(si.on_wait):
                        d = mb.InstDrain(
                            name=f"{ins.name}-ws{n}", ins=[], outs=[])
                        d.engine = ins.engine
                        d.sync_info = mb.SyncInfo(on_wait=[w], on_update=[])
                        new_list.append(d)
                        n += 1
                    ins.sync_info = mb.SyncInfo(
                        on_wait=keep, on_update=list(si.on_update))
                new_list.append(ins)
            del insts[:]
            insts.extend(new_list)
    return n


def _prep_host(inputs):
    emb = np.asarray(inputs["emb"], np.float32)
    T = np.asarray(inputs["transitions"], np.float32)
    W1 = np.asarray(inputs["W1"], np.float32)
    b1 = np.asarray(inputs["b1"], np.float32)
    W2 = np.asarray(inputs["W2"], np.float32)
    b2 = np.asarray(inputs["b2"], np.float32)

    emb_pad = np.zeros((V, EPAD), np.float32)
    emb_pad[:, 0:I] = emb
    emb_pad[:, I] = 1.0  # bias-aug ones row

    # gate reorder: pytorch [i,f,g,o] -> ours [f,i,g,o]
    perm = np.concatenate([np.arange(I, 2 * I), np.arange(0, I),
                           np.arange(2 * I, 3 * I), np.arange(3 * I, 4 * I)])

    def pack_dir(Wih, Whh, bih, bhh):
        Wih, Whh = Wih[perm].copy(), Whh[perm].copy()
        bias = (bih + bhh)[perm].copy()
        # 2x the g-gate pre-acts: kernel computes tanh(g) as 2*sigma(2g)-1
        Wih[2 * I:3 * I] *= 2.0
        Whh[2 * I:3 * I] *= 2.0
        bias[2 * I:3 * I] *= 2.0
        wih = np.zeros((4, 128, 128), np.float32)  # [gamma, k, m]
        whh = np.zeros((4, 128, 128), np.float32)
        for g in range(4):
            wih[g, 0:I, 0:I] = Wih[g * I:(g + 1) * I].T
            wih[g, I, 0:I] = bias[g * I:(g + 1) * I]
            whh[g, 0:I, 0:I] = Whh[g * I:(g + 1) * I].T
        return wih, whh

    wih_f, whh_f = pack_dir(np.asarray(inputs["Wih_f"], np.float32),
                            np.asarray(inputs["Whh_f"], np.float32),
                            np.asarray(inputs["bih_f"], np.float32),
                            np.asarray(inputs["bhh_f"], np.float32))
    wih_b, whh_b = pack_dir(np.asarray(inputs["Wih_b"], np.float32),
                            np.asarray(inputs["Whh_b"], np.float32),
                            np.asarray(inputs["bih_b"], np.float32),
                            np.asarray(inputs["bhh_b"], np.float32))

    wih = np.zeros((128, GCH, 128), np.float32)
    whh = np.zeros((128, GCH, 128), np.float32)
    for g in range(4):
        wih[:, g * 2 + 0, :] = wih_f[g]
        wih[:, g * 2 + 1, :] = wih_b[g]
        whh[:, g * 2 + 0, :] = whh_f[g]
        whh[:, g * 2 + 1, :] = whh_b[g]

    Wc = W2 @ W1                      # [19, 200]
    bcv = W2 @ b1 + b2                # [19]
    wc = np.zeros((128, 2, NB), np.float32)
    wc[0:I, 0, :] = Wc[:, 0:I].T
    wc[0:I, 1, :] = Wc[:, I:2 * I].T

    c0 = float(np.log(np.sum(np.exp(bcv))))
    esm = np.exp(T - c0)
    est = esm.T.copy()
    etb = np.exp(T[BOS, :]).reshape(NB, 1)
    veb = np.broadcast_to(np.exp(T[:, EOS]).reshape(NB, 1), (NB, BC)).copy()

    bf = ml_dtypes.bfloat16
    common = {
        "emb_pad": emb_pad.astype(bf),
        "wih": wih.astype(bf),
        "whh": whh.astype(bf),
        "wc": wc.astype(bf),
        "bc": bcv.reshape(NB, 1).astype(np.float32),
        "esm": esm.astype(bf),
        "est": est.astype(bf),
        "etb": etb.astype(np.float32),
        "veb": veb.astype(np.float32),
        "ones19": np.ones((NB, 1), bf),
        "ones19f": np.ones((NB, 1), np.float32),
        "one1x19": np.ones((1, NB), bf),
    }
    return common, c0, bcv


def kernel(**inputs):
    x = np.asarray(inputs["x"]).reshape(B, S).astype(np.int64)
    target = np.asarray(inputs["target"]).reshape(B, S).astype(np.int64)
    T = np.asarray(inputs["transitions"], np.float32)

    common, c0, bcv = _prep_host(inputs)

    common["eye"] = np.eye(128, dtype=ml_dtypes.bfloat16)
    in_maps = []
    for c in range(NCORES):
        xs = x[c * BC:(c + 1) * BC]  # [BC, S]
        # fwd token order: col t*BC + b  -> x[b, t]
        idx_fwd = xs.T.reshape(-1).astype(np.int32)
        idxs = idx_fwd.reshape(NT // 128, 128).T.copy()
        in_maps.append({**common, "idxs": idxs})

    if "nc" not in _CACHE:
        nc0 = _build_nc()
        _split_waits(nc0)
        mybir.codegen_inst_isa_subclasses(nc0)
        _CACHE["nc"] = nc0
    nc = _CACHE["nc"]
    _CACHE["in_maps"] = in_maps

    results = run_bass_kernel_spmd(nc, in_maps, list(range(NCORES))).results

    # host combine
    t_sc = (T[target[:, :-1], target[:, 1:]].sum(1)
            + T[BOS, target[:, 0]] + T[target[:, -1], EOS])  # [B]

    losses = np.zeros(B, np.float64)
    for c in range(NCORES):
        yv = np.asarray(results[c]["y_out"], ml_dtypes.bfloat16).astype(np.float32)
        res = np.asarray(results[c]["res"], np.float32)
        logY = np.log(yv).reshape(NB, S, BC)  # log Y = em + bc - c0... (em+bc)
        tg = target[c * BC:(c + 1) * BC]      # [BC, S]
        bi = np.arange(BC)
        e_sc = np.zeros(BC, np.float64)
        for t in range(S):
            e_sc += logY[tg[:, t], t, bi]
        partition = res[0] + res[1] + res[2] + (S - 1) * c0
        losses[c * BC:(c + 1) * BC] = (
            e_sc + t_sc[c * BC:(c + 1) * BC] - partition
        )
    return np.float32(-losses.mean())



# revision 14
# speedup vs baseline: 1.1824x; 1.0825x over previous
"""BiLSTM+CRF NLL loss kernel for 8 Trainium2 NeuronCores (v3).

Sharding: data-parallel on batch (32 sequences per core). Each core runs the
full BiLSTM + emission + CRF forward/backward partition recurrences for its
shard; host combines per-core partials into the scalar loss.

v3 vs v2 (trace: 2551ns/step chain = mm,σ,tanh_g,mul,add,tanh_c,mul_h):
- tanh(g) removed from the serial ACT chain: g pre-acts are scaled 2x on the
  host so one sigmoid instruction covers f,i,g (tanh(g) = 2σ(2g)-1), and the
  cell update becomes c = f*c + 2(σ_g-0.5)*i via fused scalar_tensor_tensor
- cell/gate DVE pipeline in bf16 (2x DVE rate), cell updated in place in a
  fixed tile (no cross-engine WAR)
- CRF fwd/bwd hop multiplies merged into one strided-AP tensor_tensor
"""

import numpy as np
import ml_dtypes

import concourse.bass as bass
import concourse.tile as tile
from concourse import mybir
from concourse.bass_utils import run_bass_kernel_spmd

F32 = mybir.dt.float32
BF16 = mybir.dt.bfloat16

B, S, V, I, NB = 256, 512, 30000, 100, 19
BOS, EOS = 17, 18
NCORES = 8
BC = B // NCORES          # 32 sequences per core
NT = BC * S               # 16384 tokens per core
KP = I + 1                # 101: embedding dims + ones row (bias aug)
EPAD = 128                # padded embedding row length
RENORM = 16               # CRF renorm interval
TBLK = 4                  # steps per PSUM gate block
GCH = 8                   # gate chunks: (gamma in [g,f,i,o]) x (dir in [f,b])

_CACHE = {}


def _build_nc(s_len=S):
    SL = s_len
    NTL = BC * SL
    NBLK = SL // TBLK
    NCHUNK = NTL // 128

    nc = bass.Bass()

    # ---- dram I/O ----
    emb_d = nc.dram_tensor("emb_pad", [V, EPAD], BF16, kind="ExternalInput")
    idx_d = nc.dram_tensor("idxs", [128, NTL // 128], mybir.dt.int32, kind="ExternalInput")
    eye_d = nc.dram_tensor("eye", [128, 128], BF16, kind="ExternalInput")
    wih_d = nc.dram_tensor("wih", [128, GCH, 128], BF16, kind="ExternalInput")
    whh_d = nc.dram_tensor("whh", [128, GCH, 128], BF16, kind="ExternalInput")
    wc_d = nc.dram_tensor("wc", [128, 2, NB], BF16, kind="ExternalInput")
    bc_d = nc.dram_tensor("bc", [NB, 1], F32, kind="ExternalInput")
    esm_d = nc.dram_tensor("esm", [NB, NB], BF16, kind="ExternalInput")
    est_d = nc.dram_tensor("est", [NB, NB], BF16, kind="ExternalInput")
    etb_d = nc.dram_tensor("etb", [NB, 1], F32, kind="ExternalInput")
    veb_d = nc.dram_tensor("veb", [NB, BC], F32, kind="ExternalInput")
    ones19_d = nc.dram_tensor("ones19", [NB, 1], BF16, kind="ExternalInput")
    ones19f_d = nc.dram_tensor("ones19f", [NB, 1], F32, kind="ExternalInput")
    one1x19_d = nc.dram_tensor("one1x19", [1, NB], BF16, kind="ExternalInput")

    y_out = nc.dram_tensor("y_out", [NB, NTL], BF16, kind="ExternalOutput")
    res_out = nc.dram_tensor("res", [4, BC], F32, kind="ExternalOutput")

    SIG = mybir.ActivationFunctionType.Sigmoid
    TANH = mybir.ActivationFunctionType.Tanh
    EXP = mybir.ActivationFunctionType.Exp
    LOG = mybir.ActivationFunctionType.Ln

    with tile.TileContext(nc) as tc:
        with tc.tile_pool(name="big", bufs=1) as bp:
            xeT_f = bp.tile([128, NTL], BF16, tag="xeT_f")
            eye_s = bp.tile([128, 128], BF16, tag="eye_s")
            # h storage: col (t+1)*32 = h after step t; col 0 = h(-1)=0
            h_all = bp.tile([128, 2, NTL + BC], BF16, tag="h_all")
            Y = bp.tile([NB, NTL], BF16, tag="Y")
            idx_f = bp.tile([128, NTL // 128], mybir.dt.int32, tag="idx_f")
            wih = bp.tile([128, GCH, 128], BF16, tag="wih")
            whh = bp.tile([128, GCH, 128], BF16, tag="whh")
            wc = bp.tile([128, 2, NB], BF16, tag="wc")
            bc_s = bp.tile([NB, 1], F32, tag="bc_s")
            esm = bp.tile([NB, NB], BF16, tag="esm")
            est = bp.tile([NB, NB], BF16, tag="est")
            etb = bp.tile([NB, 1], F32, tag="etb")
            veb = bp.tile([NB, BC], F32, tag="veb")
            ones19 = bp.tile([NB, 1], BF16, tag="ones19")
            ones19f = bp.tile([NB, 1], F32, tag="ones19f")
            one1x19 = bp.tile([1, NB], BF16, tag="one1x19")
            gates_s0 = bp.tile([128, GCH, BC], BF16, tag="gates_s0")
            gates_s1 = bp.tile([128, GCH, BC], BF16, tag="gates_s1")
            cellc = bp.tile([128, 2, BC], BF16, tag="cellc")  # c, in-place
            th0 = bp.tile([128, 2, BC], BF16, tag="th0")
            th1 = bp.tile([128, 2, BC], BF16, tag="th1")
            u_f = bp.tile([128, 2, BC], BF16, tag="u_f")
            u_i = bp.tile([128, 2, BC], BF16, tag="u_i")
            # CRF chain state: [parity, chain(0=fwd,1=bwd), BC]
            CH = bp.tile([NB, 2, 2, BC], BF16, tag="CH")
            P2 = bp.tile([NB, BC], F32, tag="P2")
            acc_f = bp.tile([1, BC], F32, tag="acc_f")
            acc_b = bp.tile([1, BC], F32, tag="acc_b")
            rec_f = bp.tile([1, BC], F32, tag="rec_f")
            rec_fb = bp.tile([1, BC], BF16, tag="rec_fb")
            rec_b = bp.tile([1, BC], F32, tag="rec_b")
            rec_bb = bp.tile([1, BC], BF16, tag="rec_bb")
            lg_f = bp.tile([1, BC], F32, tag="lg_f")
            lg_b = bp.tile([1, BC], F32, tag="lg_b")
            res_s = bp.tile([4, BC], F32, tag="res_s")

            # ---- small loads ----
            nc.sync.dma_start(out=idx_f[:, :], in_=idx_d[:])
            nc.sync.dma_start(out=eye_s[:, :], in_=eye_d[:])
            nc.sync.dma_start(out=wih[:, :, :], in_=wih_d[:])
            nc.sync.dma_start(out=whh[:, :, :], in_=whh_d[:])
            nc.sync.dma_start(out=wc[:, :, :], in_=wc_d[:])
            nc.sync.dma_start(out=bc_s[:, :], in_=bc_d[:])
            nc.sync.dma_start(out=esm[:, :], in_=esm_d[:])
            nc.sync.dma_start(out=est[:, :], in_=est_d[:])
            nc.sync.dma_start(out=etb[:, :], in_=etb_d[:])
            nc.sync.dma_start(out=veb[:, :], in_=veb_d[:])
            nc.sync.dma_start(out=ones19[:, :], in_=ones19_d[:])
            nc.sync.dma_start(out=ones19f[:, :], in_=ones19f_d[:])
            nc.sync.dma_start(out=one1x19[:, :], in_=one1x19_d[:])

            nc.vector.memset(acc_f[:, :], 0.0)
            nc.vector.memset(acc_b[:, :], 0.0)

            # =========== phase A: gather + both LSTMs, interleaved ==========
            # one nat tile per chunk: gather DMAs then carry no pool-WAR
            # waits, so no multi-us gpsimd Drains throttle the gather
            with tc.tile_pool(name="gat", bufs=NCHUNK) as gp, \
                 tc.tile_pool(name="gps", bufs=2, space="PSUM") as gpp, \
                 tc.tile_pool(name="psA", bufs=1, space="PSUM") as pa:
                GA = pa.tile([128, GCH, TBLK, BC], F32, tag="GA")
                GB = pa.tile([128, GCH, TBLK, BC], F32, tag="GB")
                gbuf = (GA, GB)

                nat_tiles = {}

                def gather_dma(c):
                    nat = gp.tile([128, EPAD], BF16, tag="nat")
                    nc.gpsimd.indirect_dma_start(
                        out=nat[:, :], out_offset=None,
                        in_=emb_d[:, :],
                        in_offset=bass.IndirectOffsetOnAxis(
                            ap=idx_f[:, c:c + 1], axis=0),
                    )
                    nat_tiles[c] = nat

                def gather_tp(c):
                    nat = nat_tiles.pop(c)
                    tp = gpp.tile([128, 128], BF16, tag="tp")
                    nc.tensor.transpose(tp[:, :], nat[:, :], eye_s[:, :])
                    nc.vector.tensor_copy(
                        xeT_f[:, c * 128:(c + 1) * 128], tp[:, :])

                xe3 = xeT_f[0:KP, :].rearrange("p (t b) -> p t b", b=BC)

                def bulk_mm(k, c):
                    G = gbuf[k % 2]
                    if c % 2 == 0:
                        rhs = xe3[:, k * TBLK:(k + 1) * TBLK, :]
                    else:
                        hi = SL - 1 - k * TBLK
                        rhs = (xe3[:, hi:hi - TBLK:-1, :]
                               if hi - TBLK >= 0 else xe3[:, hi::-1, :])
                    nc.tensor.matmul(
                        G[:, c, :, :], wih[0:KP, c, :], rhs,
                        start=True, stop=False, skip_group_check=True,
                    )

                MULT = mybir.AluOpType.mult
                ADD = mybir.AluOpType.add
                SUB = mybir.AluOpType.subtract

                def step(t):
                    G = gbuf[(t // TBLK) % 2]
                    tau = t % TBLK
                    rd = t * BC
                    gs = gates_s0 if t % 2 == 0 else gates_s1
                    th = th0 if t % 2 == 0 else th1
                    if t > 0:
                        # recurrent matmuls: f,i,g first, then o
                        for c in (0, 1, 2, 3, 4, 5):
                            d = c % 2
                            nc.tensor.matmul(
                                G[:, c, tau, :], whh[0:I, c, :],
                                h_all[0:I, d, rd:rd + BC],
                                start=False, stop=True, skip_group_check=True,
                            )
                    # one sigmoid covers f,i,g (g pre-acts carry a 2x host
                    # scale, so sigma here encodes tanh(g) = 2*sigma(2g)-1)
                    nc.scalar.activation(gs[:, 0:6, :], G[:, 0:6, tau, :], SIG)
                    if t > 0:
                        for c in (6, 7):
                            d = c % 2
                            nc.tensor.matmul(
                                G[:, c, tau, :], whh[0:I, c, :],
                                h_all[0:I, d, rd:rd + BC],
                                start=False, stop=True, skip_group_check=True,
                            )
                    # sigmoid(o) off the critical path
                    nc.scalar.activation(gs[:, 6:8, :], G[:, 6:8, tau, :], SIG)
                    # u_i = (sigma_g - 0.5) * i   [= tanh(g)/2 * i]
                    nc.vector.scalar_tensor_tensor(
                        u_i[:, :, :], gs[:, 4:6, :], 0.5, gs[:, 2:4, :],
                        op0=SUB, op1=MULT)
                    if t > 0:
                        # u_f = f * c ; c = 2*u_i + u_f  (in place)
                        # u_f runs on Pool once the gathers are done: the DVE
                        # chain shrinks to STT(u_i) -> STT(c) while Pool
                        # computes u_f concurrently
                        ue = nc.gpsimd if t >= 244 else nc.vector
                        ue.tensor_mul(
                            u_f[:, :, :], gs[:, 0:2, :], cellc[:, :, :])
                        nc.vector.scalar_tensor_tensor(
                            cellc[:, :, :], u_i[:, :, :], 2.0, u_f[:, :, :],
                            op0=MULT, op1=ADD)
                    else:
                        nc.vector.tensor_scalar_mul(
                            cellc[:, :, :], u_i[:, :, :], 2.0)
                    nc.scalar.activation(th[:, :, :], cellc[:, :, :], TANH)
                    wr = (t + 1) * BC
                    nc.vector.tensor_mul(
                        h_all[:, :, wr:wr + BC], gs[:, 6:8, :], th[:, :, :]
                    )

                import os
                _ALLGATHER = bool(int(os.environ.get("KV2_ALLGATHER", "0")))
                if _ALLGATHER:
                    for j in range(NCHUNK):
                        gather_dma(j)
                        gather_tp(j)
                else:
                    # prologue: DMA chunks for blocks 0..5 (both ends,
                    # interleaved so early transposes unblock first),
                    # transpose chunks for blocks 0..3
                    for j in (0, 1, 2, 3, 4, 5):
                        gather_dma(j)
                        gather_dma(NCHUNK - 1 - j)
                    for j in (0, 1, 2, 3):
                        gather_tp(j)
                        gather_tp(NCHUNK - 1 - j)
                for c in range(GCH):
                    bulk_mm(0, c)
                for c in range(GCH):
                    bulk_mm(1, c)

                # chunk c serves fwd block c and bwd block NCHUNK-1-c, so every
                # chunk must be resident before block NBLK//2. DMA the chunk
                # used by block j at block j-6, transpose it at block j-4 —
                # ~2 blocks of runway so transposes never stall PE on gpsimd.
                mid = NCHUNK // 2 - 1  # 63
                for k in range(NBLK):
                    for tau in range(TBLK):
                        step(k * TBLK + tau)
                        # spread helper work across the 4 steps of the block
                        if tau == 0:
                            if not _ALLGATHER and 6 <= k + 6 <= mid:
                                gather_dma(k + 6)
                            if k + 2 < NBLK:
                                bulk_mm(k + 2, 0)
                                bulk_mm(k + 2, 2)
                        elif tau == 1:
                            if not _ALLGATHER and 4 <= k + 4 <= mid:
                                gather_tp(k + 4)
                            if k + 2 < NBLK:
                                bulk_mm(k + 2, 4)
                                bulk_mm(k + 2, 6)
                        elif tau == 2:
                            cb = NCHUNK - 7 - k
                            if not _ALLGATHER and cb >= mid + 1:
                                gather_dma(cb)
                            if k + 2 < NBLK:
                                bulk_mm(k + 2, 1)
                                bulk_mm(k + 2, 3)
                        else:
                            cb = NCHUNK - 5 - k
                            if not _ALLGATHER and mid + 1 <= cb <= NCHUNK - 5:
                                gather_tp(cb)
                            if k + 2 < NBLK:
                                bulk_mm(k + 2, 5)
                                bulk_mm(k + 2, 7)

            # ==== phase B+C: emissions Y = exp(em + bc), interleaved with
            # ==== the CRF partition chains (B blocks feed C just in time;
            # ==== exp and ln share the natural_log_exp activation table)
            EBLK = 16  # tokens per emission block
            hb_ap = h_all[0:I, 1, :].rearrange("p (t b) -> p t b", b=BC)

            with tc.tile_pool(name="psB", bufs=3, space="PSUM") as pb, \
                 tc.tile_pool(name="psC", bufs=2, space="PSUM") as pc, \
                 tc.tile_pool(name="psC2", bufs=1, space="PSUM") as pc2:

                def emit_block(blk):
                    t0 = blk * EBLK
                    em_ps = pb.tile([NB, EBLK * BC], F32, tag="em_ps")
                    HB = EBLK // 2
                    for hlf in range(2):
                        th0_ = t0 + hlf * HB
                        sl = slice(hlf * HB * BC, (hlf + 1) * HB * BC)
                        # hf for token t lives at col (t+1)*BC
                        nc.tensor.matmul(
                            em_ps[:, sl], wc[0:I, 0, :],
                            h_all[0:I, 0, (th0_ + 1) * BC:(th0_ + 1 + HB) * BC],
                            start=True, stop=False, skip_group_check=True,
                        )
                        # hb for token t lives at round (SL-1-t): col (SL-t)*BC
                        nc.tensor.matmul(
                            em_ps[:, sl].rearrange("p (t b) -> p t b", b=BC),
                            wc[0:I, 1, :],
                            hb_ap[:, SL - th0_:SL - th0_ - HB:-1, :],
                            start=False, stop=True, skip_group_check=True,
                        )
                    nc.scalar.activation(
                        Y[:, t0 * BC:(t0 + EBLK) * BC], em_ps[:, :], EXP,
                        bias=bc_s[:, 0:1]
                    )

                emit_block(0)
                emit_block(31)
                emit_block(30)
                # W0 = Y_0 * exp(T[BOS,:]) ; V = veb * Y_last
                # chain state in CH[parity, chain, :]: hop r reads parity
                # (r-1)%2, writes r%2 — no in-place WAR; the fwd and bwd hop
                # multiplies merge into ONE strided-AP tensor_tensor per r
                Yp = Y[0:NB, :].rearrange("p (t b) -> p t b", b=BC)
                nc.vector.tensor_scalar_mul(CH[:, 0, 0, :], Y[0:NB, 0:BC],
                                            etb[:, 0:1])
                nc.vector.tensor_mul(CH[:, 1, 1, :], veb[:, :],
                                     Y[0:NB, (SL - 1) * BC:SL * BC])

                # small phase-C PSUM tensors: one bank per chain so the fwd
                # and bwd renorm pipelines don't false-serialize on a bank
                crfF = pc2.tile([NB, 3 * BC], F32, tag="crfF")
                crfB = pc2.tile([NB, 2 * BC], F32, tag="crfB")
                rf_ps = crfF[:, 0:BC]
                sf_ps = crfF[0:1, BC:2 * BC]
                dot_ps = crfF[0:1, 2 * BC:3 * BC]
                rb_ps = crfB[:, 0:BC]
                sb_ps = crfB[0:1, BC:2 * BC]

                def renorm_snap(w_sb, s_ps):
                    # s = ones19^T @ w (PE, off the recurrence chain)
                    nc.tensor.matmul(s_ps, ones19[:, :], w_sb[:, :],
                                     skip_group_check=True)

                def renorm_mid(s_ps, rec, recb, r_ps, lg, acc):
                    nc.vector.reciprocal(rec[:, :], s_ps)
                    nc.vector.tensor_copy(recb[:, :], rec[:, :])
                    nc.tensor.matmul(r_ps, one1x19[:, :], recb[:, :],
                                     skip_group_check=True)
                    nc.scalar.activation(lg[:, :], s_ps, LOG)
                    nc.vector.tensor_add(acc[:, :], acc[:, :], lg[:, :])

                wb_prev = None
                HALF = SL // 2
                for r in range(HALF):
                    p, q = r % 2, (r + 1) % 2  # dst / src parity
                    ty = SL - 2 - r  # next Y column for backward chain
                    # just-in-time emission production: low block k+1 at
                    # r=16k+2, high block 29-k at r=16k+9 (k = 0..14/13;
                    # blocks 0, 31, 30 are produced before the loop)
                    if r % RENORM == 2 and r // RENORM <= 14:
                        emit_block(r // RENORM + 1)
                    elif r % RENORM == 9 and r // RENORM <= 13:
                        emit_block(29 - r // RENORM)
                    wfb_ps = pc.tile([NB, 2, BC], F32, tag="wfb_ps")
                    # backward chain mm (always)
                    nc.tensor.matmul(wfb_ps[:, 1, :], est[:, :],
                                     CH[:, q, 1, :], skip_group_check=True)
                    # forward chain mm: t = r = 1..HALF-1
                    if r >= 1:
                        nc.tensor.matmul(wfb_ps[:, 0, :], esm[:, :],
                                         CH[:, q, 0, :], skip_group_check=True)
                    if r % RENORM == 0 and r >= RENORM:
                        renorm_snap(CH[:, q, 0, :], sf_ps)
                        renorm_snap(CH[:, q, 1, :], sb_ps)
                    # hop multiplies: one strided TT covers both chains
                    if 1 <= r < HALF - 1:
                        nc.vector.tensor_mul(
                            CH[:, p, :, :], wfb_ps[:, :, :],
                            Yp[:, r:ty + 1:(ty - r), :])
                    elif r == 0:
                        nc.vector.tensor_mul(
                            CH[:, p, 1, :], wfb_ps[:, 1, :],
                            Y[0:NB, ty * BC:(ty + 1) * BC])
                    else:  # r == HALF-1: forward hop only
                        nc.vector.tensor_mul(
                            CH[:, p, 0, :], wfb_ps[:, 0, :],
                            Y[0:NB, r * BC:(r + 1) * BC])
                    if r % RENORM == 0 and r >= RENORM:
                        renorm_mid(sf_ps, rec_f, rec_fb, rf_ps, lg_f, acc_f)
                        renorm_mid(sb_ps, rec_b, rec_bb, rb_ps, lg_b, acc_b)
                    if r >= RENORM + 3 and (r - 3) % RENORM == 0:
                        nc.vector.tensor_mul(CH[:, p, 0, :], CH[:, p, 0, :],
                                             rf_ps)
                        nc.vector.tensor_mul(CH[:, p, 1, :], CH[:, p, 1, :],
                                             rb_ps)
                    wb_prev = wfb_ps

                # meet at t=HALF-1: P2 = Wf_{HALF-1} * beta_{HALF-1}
                nc.vector.tensor_mul(P2[:, :], CH[:, (HALF - 1) % 2, 0, :],
                                     wb_prev[:, 1, :])
                nc.tensor.matmul(dot_ps, ones19f[:, :], P2[:, :],
                                 skip_group_check=True)
                nc.scalar.activation(res_s[0:1, :], dot_ps, LOG)

            nc.sync.dma_start(out=y_out[:], in_=Y[:, :])
            nc.sync.dma_start(out=res_out[0:1], in_=res_s[0:1, :])
            nc.sync.dma_start(out=res_out[1:2], in_=acc_f[:, :])
            nc.sync.dma_start(out=res_out[2:3], in_=acc_b[:, :])

    return nc


def _split_waits(nc):
    """Walrus codegen allows ~1 sync-wait on compute instrs; move excess
    waits onto injected same-engine Drain instructions (which allow many).

    Keep the wait most likely to be satisfied LAST inline on the compute
    instruction (a cross-engine producer), and drain the early-satisfied
    ones (same-engine program-order waits) first — a drain blocked on the
    critical producer adds ~70-90ns of serial queue decode vs an inline
    wait that fires as soon as the semaphore lands."""
    from concourse import mybir as mb

    def sem_engine(w):
        nm = getattr(w, 'ant_name', '') or ''
        return nm.split('_')[0]

    eng_name = {
        mb.EngineType.PE: 'PE', mb.EngineType.Activation: 'Activation',
        mb.EngineType.DVE: 'DVE', mb.EngineType.Pool: 'Pool',
        mb.EngineType.SP: 'SP',
    }
    n = 0
    for f in nc.m.functions:
        for blk in f.blocks:
            insts = blk.instructions
            new_list = []
            for ins in insts:
                si = ins.sync_info
                if si is not None and si.on_wait and len(si.on_wait) > 1:
                    waits = list(si.on_wait)
                    own = eng_name.get(ins.engine, '?')
                    cross = [w for w in waits if sem_engine(w) != own]
                    selfw = [w for w in waits if sem_engine(w) == own]
                    inline = [cross[-1]] if cross else [waits[-1]]
                    rest = [w for w in waits if w is not inline[0]]
                    # self-engine waits first (satisfied early), cross after
                    rest.sort(key=lambda w: 0 if sem_engine(w) == own else 1)
                    for w in rest:
                        d = mb.InstDrain(
                            name=f"{ins.name}-ws{n}", ins=[], outs=[])
                        d.engine = ins.engine
                        d.sync_info = mb.SyncInfo(on_wait=[w], on_update=[])
                        new_list.append(d)
                        n += 1
                    ins.sync_info = mb.SyncInfo(
                        on_wait=inline, on_update=list(si.on_update))
                new_list.append(ins)
            del insts[:]
            insts.extend(new_list)
    return n


def _prep_host(inputs):
    emb = np.asarray(inputs["emb"], np.float32)
    T = np.asarray(inputs["transitions"], np.float32)
    W1 = np.asarray(inputs["W1"], np.float32)
    b1 = np.asarray(inputs["b1"], np.float32)
    W2 = np.asarray(inputs["W2"], np.float32)
    b2 = np.asarray(inputs["b2"], np.float32)

    emb_pad = np.zeros((V, EPAD), np.float32)
    emb_pad[:, 0:I] = emb
    emb_pad[:, I] = 1.0  # bias-aug ones row

    # gate reorder: pytorch [i,f,g,o] -> ours [f,i,g,o]
    perm = np.concatenate([np.arange(I, 2 * I), np.arange(0, I),
                           np.arange(2 * I, 3 * I), np.arange(3 * I, 4 * I)])

    def pack_dir(Wih, Whh, bih, bhh):
        Wih, Whh = Wih[perm].copy(), Whh[perm].copy()
        bias = (bih + bhh)[perm].copy()
        # 2x the g-gate pre-acts: kernel computes tanh(g) as 2*sigma(2g)-1
        Wih[2 * I:3 * I] *= 2.0
        Whh[2 * I:3 * I] *= 2.0
        bias[2 * I:3 * I] *= 2.0
        wih = np.zeros((4, 128, 128), np.float32)  # [gamma, k, m]
        whh = np.zeros((4, 128, 128), np.float32)
        for g in range(4):
            wih[g, 0:I, 0:I] = Wih[g * I:(g + 1) * I].T
            wih[g, I, 0:I] = bias[g * I:(g + 1) * I]
            whh[g, 0:I, 0:I] = Whh[g * I:(g + 1) * I].T
        return wih, whh

    wih_f, whh_f = pack_dir(np.asarray(inputs["Wih_f"], np.float32),
                            np.asarray(inputs["Whh_f"], np.float32),
                            np.asarray(inputs["bih_f"], np.float32),
                            np.asarray(inputs["bhh_f"], np.float32))
    wih_b, whh_b = pack_dir(np.asarray(inputs["Wih_b"], np.float32),
                            np.asarray(inputs["Whh_b"], np.float32),
                            np.asarray(inputs["bih_b"], np.float32),
                            np.asarray(inputs["bhh_b"], np.float32))

    wih = np.zeros((128, GCH, 128), np.float32)
    whh = np.zeros((128, GCH, 128), np.float32)
    for g in range(4):
        wih[:, g * 2 + 0, :] = wih_f[g]
        wih[:, g * 2 + 1, :] = wih_b[g]
        whh[:, g * 2 + 0, :] = whh_f[g]
        whh[:, g * 2 + 1, :] = whh_b[g]

    Wc = W2 @ W1                      # [19, 200]
    bcv = W2 @ b1 + b2                # [19]
    wc = np.zeros((128, 2, NB), np.float32)
    wc[0:I, 0, :] = Wc[:, 0:I].T
    wc[0:I, 1, :] = Wc[:, I:2 * I].T

    c0 = float(np.log(np.sum(np.exp(bcv))))
    esm = np.exp(T - c0)
    est = esm.T.copy()
    etb = np.exp(T[BOS, :]).reshape(NB, 1)
    veb = np.broadcast_to(np.exp(T[:, EOS]).reshape(NB, 1), (NB, BC)).copy()

    bf = ml_dtypes.bfloat16
    common = {
        "emb_pad": emb_pad.astype(bf),
        "wih": wih.astype(bf),
        "whh": whh.astype(bf),
        "wc": wc.astype(bf),
        "bc": bcv.reshape(NB, 1).astype(np.float32),
        "esm": esm.astype(bf),
        "est": est.astype(bf),
        "etb": etb.astype(np.float32),
        "veb": veb.astype(np.float32),
        "ones19": np.ones((NB, 1), bf),
        "ones19f": np.ones((NB, 1), np.float32),
        "one1x19": np.ones((1, NB), bf),
    }
    return common, c0, bcv


def kernel(**inputs):
    x = np.asarray(inputs["x"]).reshape(B, S).astype(np.int64)
    target = np.asarray(inputs["target"]).reshape(B, S).astype(np.int64)
    T = np.asarray(inputs["transitions"], np.float32)

    common, c0, bcv = _prep_host(inputs)

    common["eye"] = np.eye(128, dtype=ml_dtypes.bfloat16)
    in_maps = []
    for c in range(NCORES):
        xs = x[c * BC:(c + 1) * BC]  # [BC, S]
        # fwd token order: col t*BC + b  -> x[b, t]
        idx_fwd = xs.T.reshape(-1).astype(np.int32)
        idxs = idx_fwd.reshape(NT // 128, 128).T.copy()
        in_maps.append({**common, "idxs": idxs})

    if "nc" not in _CACHE:
        nc0 = _build_nc()
        _split_waits(nc0)
        mybir.codegen_inst_isa_subclasses(nc0)
        _CACHE["nc"] = nc0
    nc = _CACHE["nc"]
    _CACHE["in_maps"] = in_maps

    results = run_bass_kernel_spmd(nc, in_maps, list(range(NCORES))).results

    # host combine
    t_sc = (T[target[:, :-1], target[:, 1:]].sum(1)
            + T[BOS, target[:, 0]] + T[target[:, -1], EOS])  # [B]

    losses = np.zeros(B, np.float64)
    for c in range(NCORES):
        yv = np.asarray(results[c]["y_out"], ml_dtypes.bfloat16).astype(np.float32)
        res = np.asarray(results[c]["res"], np.float32)
        logY = np.log(yv).reshape(NB, S, BC)  # log Y = em + bc - c0... (em+bc)
        tg = target[c * BC:(c + 1) * BC]      # [BC, S]
        bi = np.arange(BC)
        e_sc = np.zeros(BC, np.float64)
        for t in range(S):
            e_sc += logY[tg[:, t], t, bi]
        partition = res[0] + res[1] + res[2] + (S - 1) * c0
        losses[c * BC:(c + 1) * BC] = (
            e_sc + t_sc[c * BC:(c + 1) * BC] - partition
        )
    return np.float32(-losses.mean())



# revision 18
# speedup vs baseline: 1.2309x; 1.0410x over previous
"""BiLSTM+CRF NLL loss kernel for 8 Trainium2 NeuronCores (v3).

Sharding: data-parallel on batch (32 sequences per core). Each core runs the
full BiLSTM + emission + CRF forward/backward partition recurrences for its
shard; host combines per-core partials into the scalar loss.

v3 vs v2 (trace: 2551ns/step chain = mm,σ,tanh_g,mul,add,tanh_c,mul_h):
- tanh(g) removed from the serial ACT chain: g pre-acts are scaled 2x on the
  host so one sigmoid instruction covers f,i,g (tanh(g) = 2σ(2g)-1), and the
  cell update becomes c = f*c + 2(σ_g-0.5)*i via fused scalar_tensor_tensor
- cell/gate DVE pipeline in bf16 (2x DVE rate), cell updated in place in a
  fixed tile (no cross-engine WAR)
- CRF fwd/bwd hop multiplies merged into one strided-AP tensor_tensor
"""

import numpy as np
import ml_dtypes

import concourse.bass as bass
import concourse.tile as tile
from concourse import mybir
from concourse.bass_utils import run_bass_kernel_spmd

F32 = mybir.dt.float32
BF16 = mybir.dt.bfloat16

B, S, V, I, NB = 256, 512, 30000, 100, 19
BOS, EOS = 17, 18
NCORES = 8
BC = B // NCORES          # 32 sequences per core
NT = BC * S               # 16384 tokens per core
KP = I + 1                # 101: embedding dims + ones row (bias aug)
EPAD = 128                # padded embedding row length
RENORM = 16               # CRF renorm interval
TBLK = 4                  # steps per PSUM gate block
GCH = 8                   # gate chunks: (gamma in [g,f,i,o]) x (dir in [f,b])

_CACHE = {}


def _build_nc(s_len=S):
    SL = s_len
    NTL = BC * SL
    NBLK = SL // TBLK
    NCHUNK = NTL // 128

    nc = bass.Bass()

    # ---- dram I/O ----
    emb_d = nc.dram_tensor("emb_pad", [V, EPAD], BF16, kind="ExternalInput")
    idx_d = nc.dram_tensor("idxs", [128, NTL // 128], mybir.dt.int32, kind="ExternalInput")
    eye_d = nc.dram_tensor("eye", [128, 128], BF16, kind="ExternalInput")
    wih_d = nc.dram_tensor("wih", [128, GCH, 128], BF16, kind="ExternalInput")
    whh_d = nc.dram_tensor("whh", [128, GCH, 128], BF16, kind="ExternalInput")
    wc_d = nc.dram_tensor("wc", [128, 2, NB], BF16, kind="ExternalInput")
    bc_d = nc.dram_tensor("bc", [NB, 1], F32, kind="ExternalInput")
    esm_d = nc.dram_tensor("esm", [NB, NB], BF16, kind="ExternalInput")
    est_d = nc.dram_tensor("est", [NB, NB], BF16, kind="ExternalInput")
    etb_d = nc.dram_tensor("etb", [NB, 1], F32, kind="ExternalInput")
    veb_d = nc.dram_tensor("veb", [NB, BC], F32, kind="ExternalInput")
    ones19_d = nc.dram_tensor("ones19", [NB, 1], BF16, kind="ExternalInput")
    ones19f_d = nc.dram_tensor("ones19f", [NB, 1], F32, kind="ExternalInput")
    one1x19_d = nc.dram_tensor("one1x19", [1, NB], BF16, kind="ExternalInput")

    y_out = nc.dram_tensor("y_out", [NB, NTL], BF16, kind="ExternalOutput")
    res_out = nc.dram_tensor("res", [4, BC], F32, kind="ExternalOutput")

    SIG = mybir.ActivationFunctionType.Sigmoid
    TANH = mybir.ActivationFunctionType.Tanh
    EXP = mybir.ActivationFunctionType.Exp
    LOG = mybir.ActivationFunctionType.Ln

    with tile.TileContext(nc) as tc:
        with tc.tile_pool(name="big", bufs=1) as bp:
            xeT_f = bp.tile([128, NTL], BF16, tag="xeT_f")
            eye_s = bp.tile([128, 128], BF16, tag="eye_s")
            # h storage: col (t+1)*32 = h after step t; col 0 = h(-1)=0
            h_all = bp.tile([128, 2, NTL + BC], BF16, tag="h_all")
            Y = bp.tile([NB, NTL], BF16, tag="Y")
            idx_f = bp.tile([128, NTL // 128], mybir.dt.int32, tag="idx_f")
            wih = bp.tile([128, GCH, 128], BF16, tag="wih")
            whh = bp.tile([128, GCH, 128], BF16, tag="whh")
            wc = bp.tile([128, 2, NB], BF16, tag="wc")
            bc_s = bp.tile([NB, 1], F32, tag="bc_s")
            esm = bp.tile([NB, NB], BF16, tag="esm")
            est = bp.tile([NB, NB], BF16, tag="est")
            etb = bp.tile([NB, 1], F32, tag="etb")
            veb = bp.tile([NB, BC], F32, tag="veb")
            ones19 = bp.tile([NB, 1], BF16, tag="ones19")
            ones19f = bp.tile([NB, 1], F32, tag="ones19f")
            one1x19 = bp.tile([1, NB], BF16, tag="one1x19")
            gates_s0 = bp.tile([128, GCH, BC], BF16, tag="gates_s0")
            gates_s1 = bp.tile([128, GCH, BC], BF16, tag="gates_s1")
            cellc = bp.tile([128, 2, BC], BF16, tag="cellc")  # c, in-place
            th0 = bp.tile([128, 2, BC], BF16, tag="th0")
            th1 = bp.tile([128, 2, BC], BF16, tag="th1")
            u_f = bp.tile([128, 2, BC], BF16, tag="u_f")
            u_i = bp.tile([128, 2, BC], BF16, tag="u_i")
            # CRF chain state: [parity, chain(0=fwd,1=bwd), BC]
            CH = bp.tile([NB, 2, 2, BC], BF16, tag="CH")
            P2 = bp.tile([NB, BC], F32, tag="P2")
            acc_f = bp.tile([1, BC], F32, tag="acc_f")
            acc_b = bp.tile([1, BC], F32, tag="acc_b")
            rec_f = bp.tile([1, BC], F32, tag="rec_f")
            rec_fb = bp.tile([1, BC], BF16, tag="rec_fb")
            rec_b = bp.tile([1, BC], F32, tag="rec_b")
            rec_bb = bp.tile([1, BC], BF16, tag="rec_bb")
            lg_f = bp.tile([1, BC], F32, tag="lg_f")
            lg_b = bp.tile([1, BC], F32, tag="lg_b")
            res_s = bp.tile([4, BC], F32, tag="res_s")

            # ---- small loads ----
            nc.sync.dma_start(out=idx_f[:, :], in_=idx_d[:])
            nc.sync.dma_start(out=eye_s[:, :], in_=eye_d[:])
            nc.sync.dma_start(out=wih[:, :, :], in_=wih_d[:])
            nc.sync.dma_start(out=whh[:, :, :], in_=whh_d[:])
            nc.sync.dma_start(out=wc[:, :, :], in_=wc_d[:])
            nc.sync.dma_start(out=bc_s[:, :], in_=bc_d[:])
            nc.sync.dma_start(out=esm[:, :], in_=esm_d[:])
            nc.sync.dma_start(out=est[:, :], in_=est_d[:])
            nc.sync.dma_start(out=etb[:, :], in_=etb_d[:])
            nc.sync.dma_start(out=veb[:, :], in_=veb_d[:])
            nc.sync.dma_start(out=ones19[:, :], in_=ones19_d[:])
            nc.sync.dma_start(out=ones19f[:, :], in_=ones19f_d[:])
            nc.sync.dma_start(out=one1x19[:, :], in_=one1x19_d[:])

            nc.vector.memset(acc_f[:, :], 0.0)
            nc.vector.memset(acc_b[:, :], 0.0)

            # =========== phase A: gather + both LSTMs, interleaved ==========
            # one nat tile per chunk: gather DMAs then carry no pool-WAR
            # waits, so no multi-us gpsimd Drains throttle the gather
            with tc.tile_pool(name="gat", bufs=NCHUNK) as gp, \
                 tc.tile_pool(name="gps", bufs=2, space="PSUM") as gpp, \
                 tc.tile_pool(name="psA", bufs=1, space="PSUM") as pa:
                GA = pa.tile([128, GCH, TBLK, BC], F32, tag="GA")
                GB = pa.tile([128, GCH, TBLK, BC], F32, tag="GB")
                gbuf = (GA, GB)

                nat_tiles = {}

                def gather_dma(c):
                    nat = gp.tile([128, EPAD], BF16, tag="nat")
                    nc.gpsimd.indirect_dma_start(
                        out=nat[:, :], out_offset=None,
                        in_=emb_d[:, :],
                        in_offset=bass.IndirectOffsetOnAxis(
                            ap=idx_f[:, c:c + 1], axis=0),
                    )
                    nat_tiles[c] = nat

                def gather_tp(c):
                    nat = nat_tiles.pop(c)
                    tp = gpp.tile([128, 128], BF16, tag="tp")
                    nc.tensor.transpose(tp[:, :], nat[:, :], eye_s[:, :])
                    nc.vector.tensor_copy(
                        xeT_f[:, c * 128:(c + 1) * 128], tp[:, :])

                xe3 = xeT_f[0:KP, :].rearrange("p (t b) -> p t b", b=BC)

                def bulk_mm(k, c):
                    G = gbuf[k % 2]
                    if c % 2 == 0:
                        rhs = xe3[:, k * TBLK:(k + 1) * TBLK, :]
                    else:
                        hi = SL - 1 - k * TBLK
                        rhs = (xe3[:, hi:hi - TBLK:-1, :]
                               if hi - TBLK >= 0 else xe3[:, hi::-1, :])
                    nc.tensor.matmul(
                        G[:, c, :, :], wih[0:KP, c, :], rhs,
                        start=True, stop=False, skip_group_check=True,
                    )

                MULT = mybir.AluOpType.mult
                ADD = mybir.AluOpType.add
                SUB = mybir.AluOpType.subtract

                def step(t):
                    G = gbuf[(t // TBLK) % 2]
                    tau = t % TBLK
                    rd = t * BC
                    gs = gates_s0 if t % 2 == 0 else gates_s1
                    th = th0 if t % 2 == 0 else th1
                    if t > 0:
                        # recurrent matmuls: f,i,g first, then o
                        for c in (0, 1, 2, 3, 4, 5):
                            d = c % 2
                            nc.tensor.matmul(
                                G[:, c, tau, :], whh[0:I, c, :],
                                h_all[0:I, d, rd:rd + BC],
                                start=False, stop=True, skip_group_check=True,
                            )
                    # one sigmoid covers f,i,g (g pre-acts carry a 2x host
                    # scale, so sigma here encodes tanh(g) = 2*sigma(2g)-1)
                    nc.scalar.activation(gs[:, 0:6, :], G[:, 0:6, tau, :], SIG)
                    if t > 0:
                        for c in (6, 7):
                            d = c % 2
                            nc.tensor.matmul(
                                G[:, c, tau, :], whh[0:I, c, :],
                                h_all[0:I, d, rd:rd + BC],
                                start=False, stop=True, skip_group_check=True,
                            )
                    # sigmoid(o) off the critical path
                    nc.scalar.activation(gs[:, 6:8, :], G[:, 6:8, tau, :], SIG)
                    # cellc tracks the HALF-cell d = c/2, making the cell
                    # update end in a plain add: d = f*d + (sigma_g-0.5)*i
                    # [= f*c/2 + tanh(g)/2*i]; tanh(c) = tanh(2d) via scale
                    nc.vector.scalar_tensor_tensor(
                        u_i[:, :, :], gs[:, 4:6, :], 0.5, gs[:, 2:4, :],
                        op0=SUB, op1=MULT)
                    if t > 0:
                        nc.vector.tensor_mul(
                            u_f[:, :, :], gs[:, 0:2, :], cellc[:, :, :])
                        nc.vector.tensor_add(
                            cellc[:, :, :], u_f[:, :, :], u_i[:, :, :])
                    else:
                        nc.vector.tensor_copy(cellc[:, :, :], u_i[:, :, :])
                    nc.scalar.activation(th[:, :, :], cellc[:, :, :], TANH,
                                         scale=2.0)
                    wr = (t + 1) * BC
                    nc.vector.tensor_mul(
                        h_all[:, :, wr:wr + BC], gs[:, 6:8, :], th[:, :, :]
                    )

                import os
                _ALLGATHER = bool(int(os.environ.get("KV2_ALLGATHER", "0")))
                if _ALLGATHER:
                    for j in range(NCHUNK):
                        gather_dma(j)
                        gather_tp(j)
                else:
                    # prologue: DMA chunks for blocks 0..5 (both ends,
                    # interleaved so early transposes unblock first),
                    # transpose chunks for blocks 0..3
                    for j in (0, 1, 2, 3, 4, 5):
                        gather_dma(j)
                        gather_dma(NCHUNK - 1 - j)
                    for j in (0, 1, 2, 3):
                        gather_tp(j)
                        gather_tp(NCHUNK - 1 - j)
                for c in range(GCH):
                    bulk_mm(0, c)
                for c in range(GCH):
                    bulk_mm(1, c)

                # chunk c serves fwd block c and bwd block NCHUNK-1-c, so every
                # chunk must be resident before block NBLK//2. DMA the chunk
                # used by block j at block j-6, transpose it at block j-4 —
                # ~2 blocks of runway so transposes never stall PE on gpsimd.
                mid = NCHUNK // 2 - 1  # 63
                for k in range(NBLK):
                    for tau in range(TBLK):
                        step(k * TBLK + tau)
                        # spread helper work across the 4 steps of the block
                        if tau == 0:
                            if not _ALLGATHER and 6 <= k + 6 <= mid:
                                gather_dma(k + 6)
                            if k + 2 < NBLK:
                                bulk_mm(k + 2, 0)
                                bulk_mm(k + 2, 2)
                        elif tau == 1:
                            if not _ALLGATHER and 4 <= k + 4 <= mid:
                                gather_tp(k + 4)
                            if k + 2 < NBLK:
                                bulk_mm(k + 2, 4)
                                bulk_mm(k + 2, 6)
                        elif tau == 2:
                            cb = NCHUNK - 7 - k
                            if not _ALLGATHER and cb >= mid + 1:
                                gather_dma(cb)
                            if k + 2 < NBLK:
                                bulk_mm(k + 2, 1)
                                bulk_mm(k + 2, 3)
                        else:
                            cb = NCHUNK - 5 - k
                            if not _ALLGATHER and mid + 1 <= cb <= NCHUNK - 5:
                                gather_tp(cb)
                            if k + 2 < NBLK:
                                bulk_mm(k + 2, 5)
                                bulk_mm(k + 2, 7)

            # ==== phase B+C: emissions Y = exp(em + bc), interleaved with
            # ==== the CRF partition chains (B blocks feed C just in time;
            # ==== exp and ln share the natural_log_exp activation table)
            EBLK = 16  # tokens per emission block
            hb_ap = h_all[0:I, 1, :].rearrange("p (t b) -> p t b", b=BC)

            with tc.tile_pool(name="psB", bufs=3, space="PSUM") as pb, \
                 tc.tile_pool(name="psC", bufs=2, space="PSUM") as pc, \
                 tc.tile_pool(name="psC2", bufs=1, space="PSUM") as pc2:

                em_tiles = {}
                NPART = 4
                HB = EBLK // NPART  # tokens per emission part

                def emit_part(blk, part):
                    # one quarter-block matmul pair (N=128): bounds PE
                    # head-of-line blocking of the CRF chain matmuls
                    t0 = blk * EBLK
                    if part == 0:
                        em_new = pb.tile([NB, EBLK * BC], F32, tag="em_ps")
                        em_tiles[blk] = em_new
                    em_ps = em_tiles[blk]
                    th0_ = t0 + part * HB
                    sl = slice(part * HB * BC, (part + 1) * HB * BC)
                    # hf for token t lives at col (t+1)*BC
                    nc.tensor.matmul(
                        em_ps[:, sl], wc[0:I, 0, :],
                        h_all[0:I, 0, (th0_ + 1) * BC:(th0_ + 1 + HB) * BC],
                        start=True, stop=False, skip_group_check=True,
                    )
                    # hb for token t lives at round (SL-1-t): col (SL-t)*BC
                    nc.tensor.matmul(
                        em_ps[:, sl].rearrange("p (t b) -> p t b", b=BC),
                        wc[0:I, 1, :],
                        hb_ap[:, SL - th0_:SL - th0_ - HB:-1, :],
                        start=False, stop=True, skip_group_check=True,
                    )

                def emit_exp(blk):
                    t0 = blk * EBLK
                    em_ps = em_tiles.pop(blk)
                    nc.scalar.activation(
                        Y[:, t0 * BC:(t0 + EBLK) * BC], em_ps[:, :], EXP,
                        bias=bc_s[:, 0:1]
                    )

                def emit_block(blk):
                    for prt in range(NPART):
                        emit_part(blk, prt)
                    emit_exp(blk)

                emit_block(0)
                emit_block(31)
                emit_block(30)
                # W0 = Y_0 * exp(T[BOS,:]) ; V = veb * Y_last
                # chain state in CH[parity, chain, :]: hop r reads parity
                # (r-1)%2, writes r%2 — no in-place WAR; the fwd and bwd hop
                # multiplies merge into ONE strided-AP tensor_tensor per r
                Yp = Y[0:NB, :].rearrange("p (t b) -> p t b", b=BC)
                nc.vector.tensor_scalar_mul(CH[:, 0, 0, :], Y[0:NB, 0:BC],
                                            etb[:, 0:1])
                nc.vector.tensor_mul(CH[:, 1, 1, :], veb[:, :],
                                     Y[0:NB, (SL - 1) * BC:SL * BC])

                # small phase-C PSUM tensors: one bank per chain so the fwd
                # and bwd renorm pipelines don't false-serialize on a bank
                crfF = pc2.tile([NB, 3 * BC], F32, tag="crfF")
                crfB = pc2.tile([NB, 2 * BC], F32, tag="crfB")
                rf_ps = crfF[:, 0:BC]
                sf_ps = crfF[0:1, BC:2 * BC]
                dot_ps = crfF[0:1, 2 * BC:3 * BC]
                rb_ps = crfB[:, 0:BC]
                sb_ps = crfB[0:1, BC:2 * BC]

                def renorm_snap(w_sb, s_ps):
                    # s = ones19^T @ w (PE, off the recurrence chain)
                    nc.tensor.matmul(s_ps, ones19[:, :], w_sb[:, :],
                                     skip_group_check=True)

                def renorm_mid(s_ps, rec, recb, r_ps, lg, acc):
                    nc.vector.reciprocal(rec[:, :], s_ps)
                    nc.vector.tensor_copy(recb[:, :], rec[:, :])
                    nc.tensor.matmul(r_ps, one1x19[:, :], recb[:, :],
                                     skip_group_check=True)
                    nc.scalar.activation(lg[:, :], s_ps, LOG)
                    nc.vector.tensor_add(acc[:, :], acc[:, :], lg[:, :])

                wb_prev = None
                HALF = SL // 2
                for r in range(HALF):
                    p, q = r % 2, (r + 1) % 2  # dst / src parity
                    ty = SL - 2 - r  # next Y column for backward chain
                    # just-in-time emission production, one quarter-block of
                    # matmuls per r: low block k+1 over r=16k+2..6, high
                    # block 29-k over r=16k+9..13 (blocks 0, 31, 30 pre-loop)
                    kk, jj = divmod(r, RENORM)
                    if kk <= 14 and 2 <= jj <= 5:
                        emit_part(kk + 1, jj - 2)
                    elif kk <= 14 and jj == 6:
                        emit_exp(kk + 1)
                    elif kk <= 13 and 9 <= jj <= 12:
                        emit_part(29 - kk, jj - 9)
                    elif kk <= 13 and jj == 13:
                        emit_exp(29 - kk)
                    wfb_ps = pc.tile([NB, 2, BC], F32, tag="wfb_ps")
                    # backward chain mm (always)
                    nc.tensor.matmul(wfb_ps[:, 1, :], est[:, :],
                                     CH[:, q, 1, :], skip_group_check=True)
                    # forward chain mm: t = r = 1..HALF-1
                    if r >= 1:
                        nc.tensor.matmul(wfb_ps[:, 0, :], esm[:, :],
                                         CH[:, q, 0, :], skip_group_check=True)
                    if r % RENORM == 0 and r >= RENORM:
                        renorm_snap(CH[:, q, 0, :], sf_ps)
                        renorm_snap(CH[:, q, 1, :], sb_ps)
                    # hop multiplies: one strided TT covers both chains
                    if 1 <= r < HALF - 1:
                        nc.vector.tensor_mul(
                            CH[:, p, :, :], wfb_ps[:, :, :],
                            Yp[:, r:ty + 1:(ty - r), :])
                    elif r == 0:
                        nc.vector.tensor_mul(
                            CH[:, p, 1, :], wfb_ps[:, 1, :],
                            Y[0:NB, ty * BC:(ty + 1) * BC])
                    else:  # r == HALF-1: forward hop only
                        nc.vector.tensor_mul(
                            CH[:, p, 0, :], wfb_ps[:, 0, :],
                            Y[0:NB, r * BC:(r + 1) * BC])
                    if r % RENORM == 0 and r >= RENORM:
                        renorm_mid(sf_ps, rec_f, rec_fb, rf_ps, lg_f, acc_f)
                        renorm_mid(sb_ps, rec_b, rec_bb, rb_ps, lg_b, acc_b)
                    if r >= RENORM + 3 and (r - 3) % RENORM == 0:
                        nc.vector.tensor_mul(CH[:, p, 0, :], CH[:, p, 0, :],
                                             rf_ps)
                        nc.vector.tensor_mul(CH[:, p, 1, :], CH[:, p, 1, :],
                                             rb_ps)
                    wb_prev = wfb_ps

                # meet at t=HALF-1: P2 = Wf_{HALF-1} * beta_{HALF-1}
                nc.vector.tensor_mul(P2[:, :], CH[:, (HALF - 1) % 2, 0, :],
                                     wb_prev[:, 1, :])
                nc.tensor.matmul(dot_ps, ones19f[:, :], P2[:, :],
                                 skip_group_check=True)
                nc.scalar.activation(res_s[0:1, :], dot_ps, LOG)

            nc.sync.dma_start(out=y_out[:], in_=Y[:, :])
            nc.sync.dma_start(out=res_out[0:1], in_=res_s[0:1, :])
            nc.sync.dma_start(out=res_out[1:2], in_=acc_f[:, :])
            nc.sync.dma_start(out=res_out[2:3], in_=acc_b[:, :])

    return nc


def _split_waits(nc):
    """Walrus codegen allows ~1 sync-wait on compute instrs; move excess
    waits onto injected same-engine Drain instructions (which allow many).

    Keep the wait most likely to be satisfied LAST inline on the compute
    instruction (a cross-engine producer), and drain the early-satisfied
    ones (same-engine program-order waits) first — a drain blocked on the
    critical producer adds ~70-90ns of serial queue decode vs an inline
    wait that fires as soon as the semaphore lands."""
    from concourse import mybir as mb

    def sem_engine(w):
        nm = getattr(w, 'ant_name', '') or ''
        return nm.split('_')[0]

    eng_name = {
        mb.EngineType.PE: 'PE', mb.EngineType.Activation: 'Activation',
        mb.EngineType.DVE: 'DVE', mb.EngineType.Pool: 'Pool',
        mb.EngineType.SP: 'SP',
    }
    n = 0
    for f in nc.m.functions:
        for blk in f.blocks:
            insts = blk.instructions
            new_list = []
            for ins in insts:
                si = ins.sync_info
                if si is not None and si.on_wait and len(si.on_wait) > 1:
                    waits = list(si.on_wait)
                    own = eng_name.get(ins.engine, '?')
                    cross = [w for w in waits if sem_engine(w) != own]
                    selfw = [w for w in waits if sem_engine(w) == own]
                    inline = [cross[-1]] if cross else [waits[-1]]
                    rest = [w for w in waits if w is not inline[0]]
                    # self-engine waits first (satisfied early), cross after
                    rest.sort(key=lambda w: 0 if sem_engine(w) == own else 1)
                    for w in rest:
                        d = mb.InstDrain(
                            name=f"{ins.name}-ws{n}", ins=[], outs=[])
                        d.engine = ins.engine
                        d.sync_info = mb.SyncInfo(on_wait=[w], on_update=[])
                        new_list.append(d)
                        n += 1
                    ins.sync_info = mb.SyncInfo(
                        on_wait=inline, on_update=list(si.on_update))
                new_list.append(ins)
            del insts[:]
            insts.extend(new_list)
    return n


def _prep_host(inputs):
    emb = np.asarray(inputs["emb"], np.float32)
    T = np.asarray(inputs["transitions"], np.float32)
    W1 = np.asarray(inputs["W1"], np.float32)
    b1 = np.asarray(inputs["b1"], np.float32)
    W2 = np.asarray(inputs["W2"], np.float32)
    b2 = np.asarray(inputs["b2"], np.float32)

    emb_pad = np.zeros((V, EPAD), np.float32)
    emb_pad[:, 0:I] = emb
    emb_pad[:, I] = 1.0  # bias-aug ones row

    # gate reorder: pytorch [i,f,g,o] -> ours [f,i,g,o]
    perm = np.concatenate([np.arange(I, 2 * I), np.arange(0, I),
                           np.arange(2 * I, 3 * I), np.arange(3 * I, 4 * I)])

    def pack_dir(Wih, Whh, bih, bhh):
        Wih, Whh = Wih[perm].copy(), Whh[perm].copy()
        bias = (bih + bhh)[perm].copy()
        # 2x the g-gate pre-acts: kernel computes tanh(g) as 2*sigma(2g)-1
        Wih[2 * I:3 * I] *= 2.0
        Whh[2 * I:3 * I] *= 2.0
        bias[2 * I:3 * I] *= 2.0
        wih = np.zeros((4, 128, 128), np.float32)  # [gamma, k, m]
        whh = np.zeros((4, 128, 128), np.float32)
        for g in range(4):
            wih[g, 0:I, 0:I] = Wih[g * I:(g + 1) * I].T
            wih[g, I, 0:I] = bias[g * I:(g + 1) * I]
            whh[g, 0:I, 0:I] = Whh[g * I:(g + 1) * I].T
        return wih, whh

    wih_f, whh_f = pack_dir(np.asarray(inputs["Wih_f"], np.float32),
                            np.asarray(inputs["Whh_f"], np.float32),
                            np.asarray(inputs["bih_f"], np.float32),
                            np.asarray(inputs["bhh_f"], np.float32))
    wih_b, whh_b = pack_dir(np.asarray(inputs["Wih_b"], np.float32),
                            np.asarray(inputs["Whh_b"], np.float32),
                            np.asarray(inputs["bih_b"], np.float32),
                            np.asarray(inputs["bhh_b"], np.float32))

    wih = np.zeros((128, GCH, 128), np.float32)
    whh = np.zeros((128, GCH, 128), np.float32)
    for g in range(4):
        wih[:, g * 2 + 0, :] = wih_f[g]
        wih[:, g * 2 + 1, :] = wih_b[g]
        whh[:, g * 2 + 0, :] = whh_f[g]
        whh[:, g * 2 + 1, :] = whh_b[g]

    Wc = W2 @ W1                      # [19, 200]
    bcv = W2 @ b1 + b2                # [19]
    wc = np.zeros((128, 2, NB), np.float32)
    wc[0:I, 0, :] = Wc[:, 0:I].T
    wc[0:I, 1, :] = Wc[:, I:2 * I].T

    c0 = float(np.log(np.sum(np.exp(bcv))))
    esm = np.exp(T - c0)
    est = esm.T.copy()
    etb = np.exp(T[BOS, :]).reshape(NB, 1)
    veb = np.broadcast_to(np.exp(T[:, EOS]).reshape(NB, 1), (NB, BC)).copy()

    bf = ml_dtypes.bfloat16
    common = {
        "emb_pad": emb_pad.astype(bf),
        "wih": wih.astype(bf),
        "whh": whh.astype(bf),
        "wc": wc.astype(bf),
        "bc": bcv.reshape(NB, 1).astype(np.float32),
        "esm": esm.astype(bf),
        "est": est.astype(bf),
        "etb": etb.astype(np.float32),
        "veb": veb.astype(np.float32),
        "ones19": np.ones((NB, 1), bf),
        "ones19f": np.ones((NB, 1), np.float32),
        "one1x19": np.ones((1, NB), bf),
    }
    return common, c0, bcv


def kernel(**inputs):
    x = np.asarray(inputs["x"]).reshape(B, S).astype(np.int64)
    target = np.asarray(inputs["target"]).reshape(B, S).astype(np.int64)
    T = np.asarray(inputs["transitions"], np.float32)

    common, c0, bcv = _prep_host(inputs)

    common["eye"] = np.eye(128, dtype=ml_dtypes.bfloat16)
    in_maps = []
    for c in range(NCORES):
        xs = x[c * BC:(c + 1) * BC]  # [BC, S]
        # fwd token order: col t*BC + b  -> x[b, t]
        idx_fwd = xs.T.reshape(-1).astype(np.int32)
        idxs = idx_fwd.reshape(NT // 128, 128).T.copy()
        in_maps.append({**common, "idxs": idxs})

    if "nc" not in _CACHE:
        nc0 = _build_nc()
        _split_waits(nc0)
        mybir.codegen_inst_isa_subclasses(nc0)
        _CACHE["nc"] = nc0
    nc = _CACHE["nc"]
    _CACHE["in_maps"] = in_maps

    results = run_bass_kernel_spmd(nc, in_maps, list(range(NCORES))).results

    # host combine
    t_sc = (T[target[:, :-1], target[:, 1:]].sum(1)
            + T[BOS, target[:, 0]] + T[target[:, -1], EOS])  # [B]

    losses = np.zeros(B, np.float64)
    for c in range(NCORES):
        yv = np.asarray(results[c]["y_out"], ml_dtypes.bfloat16).astype(np.float32)
        res = np.asarray(results[c]["res"], np.float32)
        logY = np.log(yv).reshape(NB, S, BC)  # log Y = em + bc - c0... (em+bc)
        tg = target[c * BC:(c + 1) * BC]      # [BC, S]
        bi = np.arange(BC)
        e_sc = np.zeros(BC, np.float64)
        for t in range(S):
            e_sc += logY[tg[:, t], t, bi]
        partition = res[0] + res[1] + res[2] + (S - 1) * c0
        losses[c * BC:(c + 1) * BC] = (
            e_sc + t_sc[c * BC:(c + 1) * BC] - partition
        )
    return np.float32(-losses.mean())

